# revision 7
# baseline (speedup 1.0000x reference)
"""Multi-head attention (B=4, L=1024, D=1024, H=16, DH=64) on 8 TRN2 NeuronCores.

Sharding: data-parallel over batch (4) x tensor-parallel over heads (2).
Core c = 2*b + t computes, for batch b, heads [t*8, (t+1)*8):
    QT = Wq_t^T X^T, KT = Wk_t^T X^T, V = Y Wv_t        (all bf16 matmuls)
    per head: S^T = K_h Q_h^T; P^T = exp(S^T/8);
              [ctx^T; rowsum] = Vaug_h^T P^T;  ctxn = ctx / rowsum
    O_partial = ctxn^T Wo_t                              (f32, two dt-halves)
Host pre-transposes X/Y, casts to bf16, and sums the four f32 partials
(2 tensor-parallel cores x 2 dt-halves) per batch.

Engines execute their compiled instruction streams in order, so the emission
order is a hand-software-pipelined schedule: every ST (scores) step, whose exp
drain on the scalar engine is slower than the matmuls, is followed by an
independent fill chain (V projection, next d-tile QT/KT, an earlier head's
ctx, or an out-projection partial) so the tensor engine never waits for the
scalar engine to free an ST PSUM tile.

Perf notes (vs the first working version):
  - Input DMA configs are spread across four sequencers (SP/Pool/DVE/ACT);
    a single SP rail configures queues at ~0.6us each, serializing the
    input rollout and starving the PE for the first ~15us.
  - The ones-blocks of Vaug are memset with one strided op (half the data).
  - The first QT/KT drains go to the scalar engine (idle before the exps).
  - Tail: ctx tiles for the last head pair live in the (by then idle) wide
    ST PSUM pool so the out-projection chains get the full 4-slot acc pool;
    tail drains alternate scalar/vector; chain order puts both ic0 ctx
    chains first so their normalize DMA round-trips hide under ic1's PE
    work. Keeping the PE stream dense also holds it at the 2.4GHz p-state
    (it drops to 1.2GHz within ~100ns of going idle).
"""

import numpy as np
import ml_dtypes

import concourse.tile as tile
import concourse.mybir as mybir
from concourse import bacc
from concourse.bass_utils import run_bass_kernel_spmd

B, L, D, U, H = 4, 1024, 1024, 1024, 16
DH = U // H          # 64 head dim
TP = 2               # tensor-parallel ways (heads)
DL = U // TP         # 512 local units
HL = H // TP         # 8 local heads
P = 128              # partitions
NI = 512             # matmul free-dim chunk (one PSUM bank of f32)
CC = D // P          # 8 contraction chunks for projections
DT = DL // P         # 4 local d-tiles
IT = L // P          # 8 i/j tiles
NIC = L // NI        # 2 free chunks of 512
N_CORES = 8

BF16 = mybir.dt.bfloat16
F32 = mybir.dt.float32


def _build_kernel():
    nc = bacc.Bacc(
        "TRN2", target_bir_lowering=False, debug=False, num_devices=N_CORES
    )
    xt = nc.dram_tensor("xt", [D, L], BF16, kind="ExternalInput").ap()
    yt = nc.dram_tensor("yt", [D, L], BF16, kind="ExternalInput").ap()
    wq = nc.dram_tensor("wq", [D, DL], BF16, kind="ExternalInput").ap()
    wk = nc.dram_tensor("wk", [D, DL], BF16, kind="ExternalInput").ap()
    wv = nc.dram_tensor("wv", [D, DL], BF16, kind="ExternalInput").ap()
    wo = nc.dram_tensor("wo", [DL, U], BF16, kind="ExternalInput").ap()
    out_a = nc.dram_tensor("out_a", [L, U], BF16, kind="ExternalOutput").ap()
    out_c = nc.dram_tensor("out_c", [L, U], BF16, kind="ExternalOutput").ap()

    with tile.TileContext(nc) as tc:
        _mha_body(tc, out_a, out_c, xt, yt, wq, wk, wv, wo)

    nc.compile()
    return nc


def _mha_body(tc, out_a, out_c, xt, yt, wq, wk, wv, wo, dbg=None):
    nc = tc.nc
    from contextlib import ExitStack

    with ExitStack() as ctx:
        persist = ctx.enter_context(tc.tile_pool(name="persist", bufs=1))
        pt_pool = ctx.enter_context(tc.tile_pool(name="pt", bufs=4))
        # ST tiles: [P, 1024] f32 = 2 banks each
        ps_wide = ctx.enter_context(tc.tile_pool(name="ps_wide", bufs=2, space="PSUM"))
        # single-bank accumulators (projections, V, ctx, out-proj)
        ps_acc = ctx.enter_context(tc.tile_pool(name="ps_acc", bufs=4, space="PSUM"))
        small = ctx.enter_context(tc.tile_pool(name="small", bufs=4))

        # persistent SBUF tensors
        xt_sb = persist.tile([P, CC, L], BF16, tag="xt")
        yt_sb = persist.tile([P, CC, L], BF16, tag="yt")
        wq_sb = persist.tile([P, CC, DL], BF16, tag="wq")
        wk_sb = persist.tile([P, CC, DL], BF16, tag="wk")
        wv_sb = persist.tile([P, CC, DL], BF16, tag="wv")
        wo_sb = persist.tile([P, DT, U], BF16, tag="wo")
        qt_sb = persist.tile([P, DT, L], BF16, tag="qt")
        kt_sb = persist.tile([P, DT, L], BF16, tag="kt")
        # Vaug: per j-chunk, per head a 128-col block; even h: [V_h | ones],
        # odd h: [ones | V_h] (ctx^T lands on the head's own cx partitions)
        va_sb = persist.tile([P, IT, HL * P], BF16, tag="va")
        cx_sb = persist.tile([P, DT, L], BF16, tag="cx")

        # Input DMA rollout. Each dma_start costs ~0.65us of CONFIG time on
        # its issuing sequencer, serialized per rail. Keep per-cc granularity
        # only where a consumer chain pipelines per-cc (xt for QT, yt for
        # KT/V moving operand); single-config the tensors whose first
        # consumer needs all of them anyway (wk/wv: every chain contracts
        # all 8 cc; wo: needed late). wq in two halves so QT-ic0's first
        # matmuls can start before the whole 1MB lands.
        #   SP:   xt cc0..7, wo
        #   Pool: wq (2 halves), wk, wv
        #   ACT:  yt cc0..7
        wq_r = wq.rearrange("(cc p) d -> p cc d", p=P)
        wk_r = wk.rearrange("(cc p) d -> p cc d", p=P)
        wv_r = wv.rearrange("(cc p) d -> p cc d", p=P)
        xt_r = xt.rearrange("(cc p) i -> p cc i", p=P)
        yt_r = yt.rearrange("(cc p) i -> p cc i", p=P)
        nc.gpsimd.dma_start(out=wq_sb[:, 0:4], in_=wq_r[:, 0:4])
        nc.gpsimd.dma_start(out=wq_sb[:, 4:8], in_=wq_r[:, 4:8])
        nc.gpsimd.dma_start(out=wk_sb[:], in_=wk_r[:])
        nc.gpsimd.dma_start(out=wv_sb[:], in_=wv_r[:])
        for cc in range(CC):
            nc.sync.dma_start(out=xt_sb[:, cc], in_=xt_r[:, cc])
            nc.scalar.dma_start(out=yt_sb[:, cc], in_=yt_r[:, cc])
        nc.sync.dma_start(out=wo_sb[:], in_=wo.rearrange("(dt p) o -> p dt o", p=P))

        # ones-blocks of Vaug: columns [64,192) mod 256 of each j-chunk
        # (even heads keep V in the low half, odd heads in the high half).
        # One strided memset over half the tensor; the V halves are written
        # by the v_chain drains.
        va_ones = va_sb.rearrange("p it (q s) -> p it q s", s=2 * P)
        nc.vector.memset(va_ones[:, :, :, DH : DH + P], 1.0)

        scale = DH**-0.5

        # ---- chain emitters (each a short burst of independent PE work) ----

        def proj_chain(w_sb, t_sb, rhs_sb, dt, ic, copy_eng="vector"):
            ps = ps_acc.tile([P, NI], F32, tag="acc")
            for cc in range(CC):
                nc.tensor.matmul(
                    ps[:],
                    w_sb[:, cc, dt * P : (dt + 1) * P],
                    rhs_sb[:, cc, ic * NI : (ic + 1) * NI],
                    start=(cc == 0),
                    stop=(cc == CC - 1),
                )
            dst = t_sb[:, dt, ic * NI : (ic + 1) * NI]
            if copy_eng == "vector":
                nc.vector.tensor_copy(dst, ps[:])
            else:
                nc.scalar.copy(dst, ps[:])

        def v_chain(jt):
            ps = ps_acc.tile([P, NI], F32, tag="acc")
            for cc in range(CC):
                nc.tensor.matmul(
                    ps[:],
                    yt_sb[:, cc, jt * P : (jt + 1) * P],
                    wv_sb[:, cc, :],
                    start=(cc == 0),
                    stop=(cc == CC - 1),
                )
            va_blk = va_sb[:, jt].rearrange("p (h s) -> p h s", s=P)
            ps_blk = ps.rearrange("p (h s) -> p h s", s=DH)
            nc.vector.tensor_copy(va_blk[:, 0::2, 0:DH], ps_blk[:, 0::2, :])
            nc.vector.tensor_copy(va_blk[:, 1::2, DH:P], ps_blk[:, 1::2, :])

        # Deferred finishers: the normalize crosses engines (DVE -> gpsimd
        # partition_broadcast -> DVE); emitting the post-broadcast DVE ops
        # immediately would stall the in-order DVE stream (and the PSUM-
        # releasing copies queued behind it) on the gpsimd semaphore.
        # Instead each ctx chain queues them and the next fill slot flushes.
        deferred = []

        def flush_deferred():
            while deferred:
                deferred.pop(0)()

        def ctx_chain(h, ptile, ic):
            dt, r0 = divmod(h * DH, P)
            ct = ps_acc.tile([P, NI], F32, tag="acc")
            cts = ct[:]
            for jt in range(IT):
                nc.tensor.matmul(
                    cts,
                    va_sb[:, jt, h * P : (h + 1) * P],
                    ptile[:, jt, ic * NI : (ic + 1) * NI],
                    start=(jt == 0),
                    stop=(jt == IT - 1),
                )
            # The 64 rowsum rows of ct are identical copies (each ones-column
            # of Vaug reproduces the row sum), so a gpsimd partition
            # broadcast of a single row moves the rowsum to the partitions
            # the ctx rows live on — no DMA round trip. The custom DVE
            # reciprocal only works at base partition 0.
            rc = small.tile([P, NI], F32, tag="rc")
            if r0 == 0:
                # ctx in rows 0:DH, rowsum copies in rows DH:P. The gpsimd
                # broadcast source must sit at partition 0 (Q7 core 0 owns
                # partitions 0:16 and does the read), so this orientation
                # has to move the rowsum down with a SBUF->SBUF DMA.
                rs = small.tile([P, NI], F32, tag="rs")
                nc.vector.tensor_copy(rs[DH:P, :], cts[DH:P, :])
                nc.gpsimd.dma_start(out=rs[0:DH, :], in_=rs[DH:P, :])

                def fin():
                    nc.vector.reciprocal_approx_fast(rc[0:DH, :], rs[0:DH, :])
                    nc.vector.tensor_mul(
                        cx_sb[0:DH, dt, ic * NI : (ic + 1) * NI],
                        cts[0:DH, :],
                        rc[0:DH, :],
                    )
            else:
                # rowsum copies in rows 0:DH, ctx in rows DH:P: reciprocal
                # of a single row at base 0 (all DH rowsum rows are
                # identical), then gpsimd partition-broadcast (the Q7 impl
                # reads the source on core 0 and write-masks partitions
                # [0, channels) absolutely, so broadcast all 128 rows).
                nc.vector.reciprocal_approx_fast(rc[0:1, :], cts[0:1, :])
                nc.gpsimd.partition_broadcast(rc[0:P, :], rc[0:1, :])

                def fin():
                    nc.vector.tensor_mul(
                        cx_sb[DH:P, dt, ic * NI : (ic + 1) * NI],
                        cts[DH:P, :],
                        rc[DH:P, :],
                    )

            deferred.append(fin)

        def po_chain(it, oc, dts, out_ap, copy_eng="vector", po=None, dma_eng=None):
            # out-projection partial over the given d-tiles
            if po is None:
                po = ps_acc.tile([P, NI], F32, tag="acc")
            for k, dt in enumerate(dts):
                nc.tensor.matmul(
                    po[:],
                    cx_sb[:, dt, it * P : (it + 1) * P],
                    wo_sb[:, dt, oc * NI : (oc + 1) * NI],
                    start=(k == 0),
                    stop=(k == len(dts) - 1),
                )
            o_st = small.tile([P, NI], BF16, tag="ost")
            if copy_eng == "vector":
                nc.vector.tensor_copy(o_st[:], po[:])
            else:
                # scalar engine is idle once the exp stream has drained
                nc.scalar.copy(o_st[:], po[:])
            out_r = out_ap.rearrange("(it p) o -> it p o", p=P)
            dma_eng = dma_eng or nc.sync
            dma_eng.dma_start(
                out=out_r[it, :, oc * NI : (oc + 1) * NI], in_=o_st[:]
            )

        # ---- ST + exp for a head pair, fill chains between steps ----

        def st_pair(hp, fills):
            dt = hp
            ptiles = []
            for h_off in range(2):
                pt_tile = pt_pool.tile([P, IT, L], BF16, tag="pt")
                ptiles.append(pt_tile)
            fills = list(fills)
            for jt in range(IT):
                sts = [
                    ps_wide.tile([P, 2 * NI], F32, tag="wide", name=f"st{h}")
                    for h in range(2)
                ]
                # ic-outer / head-inner: the two heads' K=64 matmuls sit on
                # disjoint PE row-groups (partitions 0:64 vs 64:128), so
                # back-to-back issue lets them stream concurrently in the
                # array (tile_position auto-derives from base partitions).
                for ic in range(NIC):
                    for h_off in range(2):
                        r0 = DH * h_off
                        nc.tensor.matmul(
                            sts[h_off][:, ic * NI : (ic + 1) * NI],
                            kt_sb[r0 : r0 + DH, dt, jt * P : (jt + 1) * P],
                            qt_sb[r0 : r0 + DH, dt, ic * NI : (ic + 1) * NI],
                            start=True,
                            stop=True,
                        )
                for h_off in range(2):
                    nc.scalar.activation(
                        ptiles[h_off][:, jt, :],
                        sts[h_off][:],
                        mybir.ActivationFunctionType.Exp,
                        scale=scale,
                    )
                if jt < len(fills):
                    pending = list(deferred)
                    deferred.clear()
                    for f in fills[jt]:
                        f()
                    for f in pending:
                        f()
            return ptiles

        # ---- schedule ----
        mk = lambda f, *a: (lambda: f(*a))

        # first QT/KT drains on the scalar engine: the ACT stream is idle
        # until the first exp, and this keeps the DVE free for the va
        # memset + early v drains
        for ic in range(NIC):
            proj_chain(wq_sb, qt_sb, xt_sb, 0, ic, copy_eng="scalar")
        for ic in range(NIC):
            proj_chain(wk_sb, kt_sb, yt_sb, 0, ic, copy_eng="scalar")

        # pair 0: fill with the 8 V chains
        pt0 = st_pair(0, [[mk(v_chain, jt)] for jt in range(IT)])

        if dbg is not None:
            nc.sync.dma_start(out=dbg[3][0], in_=pt0[0][:])
            nc.sync.dma_start(out=dbg[3][1], in_=pt0[1][:])

        # QT1/KT1 ahead of pair 1 (also covers pair-0 exp tail)
        for ic in range(NIC):
            proj_chain(wq_sb, qt_sb, xt_sb, 1, ic)
        for ic in range(NIC):
            proj_chain(wk_sb, kt_sb, yt_sb, 1, ic)

        # pair 1: fill with ctx of heads 0/1 and QT2/KT2, interleaved so
        # ctx-chain PSUM tiles (whose release waits on the cross-engine
        # normalize) never claim more than every other acc slot
        pt1 = st_pair(
            1,
            [
                [mk(ctx_chain, 0, pt0[0], 0)],
                [mk(proj_chain, wq_sb, qt_sb, xt_sb, 2, 0)],
                [mk(ctx_chain, 0, pt0[0], 1)],
                [mk(proj_chain, wq_sb, qt_sb, xt_sb, 2, 1)],
                [mk(ctx_chain, 1, pt0[1], 0)],
                [mk(proj_chain, wk_sb, kt_sb, yt_sb, 2, 0)],
                [mk(ctx_chain, 1, pt0[1], 1)],
                [mk(proj_chain, wk_sb, kt_sb, yt_sb, 2, 1)],
            ],
        )

        # pair 2: fill with ctx of heads 2/3 and QT3/KT3
        pt2 = st_pair(
            2,
            [
                [mk(ctx_chain, 2, pt1[0], 0)],
                [mk(proj_chain, wq_sb, qt_sb, xt_sb, 3, 0)],
                [mk(ctx_chain, 2, pt1[0], 1)],
                [mk(proj_chain, wq_sb, qt_sb, xt_sb, 3, 1)],
                [mk(ctx_chain, 3, pt1[1], 0)],
                [mk(proj_chain, wk_sb, kt_sb, yt_sb, 3, 0)],
                [mk(ctx_chain, 3, pt1[1], 1)],
                [mk(proj_chain, wk_sb, kt_sb, yt_sb, 3, 1)],
            ],
        )

        # pair 3: fill with ctx of heads 4/5, then out-proj partial A over
        # dt 0..2 (heads 0..5 — heads 4/5 finish in this phase's early slots,
        # so the late slots can already drain 3/4 of the out-projection)
        # out DMAs alternate SP/Pool rails: a single rail's ~0.65us-per-
        # config serialization otherwise backlogs the kernel tail
        poA = [
            mk(
                po_chain, it, oc, (0, 1, 2), out_a, "vector", None,
                (nc.sync, nc.gpsimd)[(2 * it + oc) % 2],
            )
            for it in range(IT)
            for oc in range(NIC)
        ]
        # slot 7 keeps only a small fill: anything more delays the critical
        # tail ctx chains (the exp it must cover is only ~1.3us); the two
        # remaining poA chains instead plug the tail's own idle points
        pt3 = st_pair(
            3,
            [
                [mk(ctx_chain, 4, pt2[0], 0)],
                [mk(ctx_chain, 4, pt2[0], 1)],
                [mk(ctx_chain, 5, pt2[1], 0)],
                [mk(ctx_chain, 5, pt2[1], 1)],
                poA[0:4],
                poA[4:8],
                poA[8:12],
                poA[12:15],
            ],
        )

        # tail: ctx of heads 6/7 — each (head, ic) chain gets its own acc
        # tile (a shared tile serializes: the next chain's start-write waits
        # for the previous chain's pending normalize reads). ic0 chains
        # first so their normalize round-trips hide under ic1's matmuls.
        # The dt-3 out-proj chains then rotate through halves of the (by
        # now idle) wide ST tiles so the acc slots held by pending ctx
        # normalizes never gate them; drains alternate scalar/vector.
        # Tail ctx ordering: ic0 pair first (their finishers gate the first
        # out-proj chains), h6 (DMA normalize) before h7 (broadcast). The
        # ic0 finishers flush after the third chain's emission — by then
        # h6ic0's DMA round trip has completed, so nothing in the in-order
        # DVE stream blocks.
        ctx_chain(6, pt3[0], 0)
        ctx_chain(7, pt3[1], 0)
        ctx_chain(6, pt3[0], 1)
        flush_deferred()  # fins for h6ic0 and h7ic0
        ctx_chain(7, pt3[1], 1)
        # the last two out_a chains fill the window where the DVE finishers
        # for the ic0 half are still completing
        for f in poA[14:16]:
            f()
        flush_deferred()  # fins for h6ic1 and h7ic1

        def po_tile_gen():
            # [wide, wide, acc, acc] repeating: the first acc slots the po
            # chains reuse are the ic0 ctx tiles (normalized early); the ic1
            # ctx tiles only come up for reuse once their finishers have
            # run. Wide tiles are used whole (half-sharing serializes on the
            # tile-granular write-after-read hazard).
            while True:
                pw = ps_wide.tile([P, 2 * NI], F32, tag="wide", name="po_w")
                yield pw[:, 0:NI]
                pw = ps_wide.tile([P, 2 * NI], F32, tag="wide", name="po_w")
                yield pw[:, 0:NI]
                yield ps_acc.tile([P, NI], F32, tag="acc", name="po_a")
                yield ps_acc.tile([P, NI], F32, tag="acc", name="po_a")

        po_tiles = po_tile_gen()
        # it-blocks 0..3 only read the ic0-half of cx dt3, whose normalizes
        # are already flushed — emit them before the last (ic1) finishers
        # so their DMA round-trips hide behind real work
        for it in range(IT // 2):
            for oc in range(NIC):
                po_chain(it, oc, (3,), out_c,
                         copy_eng=("scalar", "vector")[oc], po=next(po_tiles),
                         dma_eng=(nc.sync, nc.gpsimd)[oc])
        for it in range(IT // 2, IT):
            for oc in range(NIC):
                po_chain(it, oc, (3,), out_c,
                         copy_eng=("scalar", "vector")[oc], po=next(po_tiles),
                         dma_eng=(nc.sync, nc.gpsimd)[oc])

        if dbg is not None:
            nc.sync.dma_start(out=dbg[0][:], in_=qt_sb[:])
            nc.sync.dma_start(out=dbg[1][:], in_=kt_sb[:])
            nc.sync.dma_start(out=dbg[2][:], in_=va_sb[:])
            nc.sync.dma_start(out=dbg[4][:], in_=cx_sb[:])


_NC_CACHE = None


def _get_nc():
    global _NC_CACHE
    if _NC_CACHE is None:
        _NC_CACHE = _build_kernel()
    return _NC_CACHE


def kernel(x, y, Wq, Wk, Wv, Wo, _trace=False):
    bf = ml_dtypes.bfloat16
    x = np.asarray(x, np.float32)
    y = np.asarray(y, np.float32)
    xtb = [np.ascontiguousarray(np.asarray(x[b]).T).astype(bf) for b in range(B)]
    ytb = [np.ascontiguousarray(np.asarray(y[b]).T).astype(bf) for b in range(B)]
    wqs = [np.ascontiguousarray(np.asarray(Wq)[:, t * DL : (t + 1) * DL]).astype(bf) for t in range(TP)]
    wks = [np.ascontiguousarray(np.asarray(Wk)[:, t * DL : (t + 1) * DL]).astype(bf) for t in range(TP)]
    wvs = [np.ascontiguousarray(np.asarray(Wv)[:, t * DL : (t + 1) * DL]).astype(bf) for t in range(TP)]
    wos = [np.ascontiguousarray(np.asarray(Wo)[t * DL : (t + 1) * DL, :]).astype(bf) for t in range(TP)]

    in_maps = []
    for b in range(B):
        for t in range(TP):
            in_maps.append(
                {
                    "xt": xtb[b],
                    "yt": ytb[b],
                    "wq": wqs[t],
                    "wk": wks[t],
                    "wv": wvs[t],
                    "wo": wos[t],
                }
            )

    nc = _get_nc()
    res = run_bass_kernel_spmd(
        nc, in_maps, core_ids=list(range(N_CORES)), trace=_trace
    )
    out = np.empty((B, L, U), np.float32)
    for b in range(B):
        out[b] = (
            np.asarray(res.results[2 * b]["out_a"], np.float32)
            + np.asarray(res.results[2 * b]["out_c"], np.float32)
            + np.asarray(res.results[2 * b + 1]["out_a"], np.float32)
            + np.asarray(res.results[2 * b + 1]["out_c"], np.float32)
        )
    if _trace:
        return out, res
    return out



# revision 17
# speedup vs baseline: 1.0023x; 1.0023x over previous
"""Multi-head attention (B=4, L=1024, D=1024, H=16, DH=64) on 8 TRN2 NeuronCores.

Sharding: data-parallel over batch (4) x tensor-parallel over heads (2).
Core c = 2*b + t computes, for batch b, heads [t*8, (t+1)*8):
    QT = Wq_t^T X^T, KT = Wk_t^T X^T, V = Y Wv_t        (all bf16 matmuls)
    per head: S^T = K_h Q_h^T; P^T = exp(S^T/8);
              [ctx^T; rowsum] = Vaug_h^T P^T;  ctxn = ctx / rowsum
    O_partial = ctxn^T Wo_t                              (f32, two dt-halves)
Host pre-transposes X/Y, casts to bf16, and sums the four f32 partials
(2 tensor-parallel cores x 2 dt-halves) per batch.

Engines execute their compiled instruction streams in order, so the emission
order is a hand-software-pipelined schedule: every ST (scores) step, whose exp
drain on the scalar engine is slower than the matmuls, is followed by an
independent fill chain (V projection, next d-tile QT/KT, an earlier head's
ctx, or an out-projection partial) so the tensor engine never waits for the
scalar engine to free an ST PSUM tile.

Perf notes (vs the first working version):
  - Input DMA configs are spread across four sequencers (SP/Pool/DVE/ACT);
    a single SP rail configures queues at ~0.6us each, serializing the
    input rollout and starving the PE for the first ~15us.
  - The ones-blocks of Vaug are memset with one strided op (half the data).
  - The first QT/KT drains go to the scalar engine (idle before the exps).
  - Tail: ctx tiles for the last head pair live in the (by then idle) wide
    ST PSUM pool so the out-projection chains get the full 4-slot acc pool;
    tail drains alternate scalar/vector; chain order puts both ic0 ctx
    chains first so their normalize DMA round-trips hide under ic1's PE
    work. Keeping the PE stream dense also holds it at the 2.4GHz p-state
    (it drops to 1.2GHz within ~100ns of going idle).
"""

import numpy as np
import ml_dtypes

import concourse.tile as tile
import concourse.mybir as mybir
from concourse import bacc
from concourse.bass_utils import run_bass_kernel_spmd

B, L, D, U, H = 4, 1024, 1024, 1024, 16
DH = U // H          # 64 head dim
TP = 2               # tensor-parallel ways (heads)
DL = U // TP         # 512 local units
HL = H // TP         # 8 local heads
P = 128              # partitions
NI = 512             # matmul free-dim chunk (one PSUM bank of f32)
CC = D // P          # 8 contraction chunks for projections
DT = DL // P         # 4 local d-tiles
IT = L // P          # 8 i/j tiles
NIC = L // NI        # 2 free chunks of 512
N_CORES = 8

BF16 = mybir.dt.bfloat16
F32 = mybir.dt.float32


def _build_kernel():
    nc = bacc.Bacc(
        "TRN2", target_bir_lowering=False, debug=False, num_devices=N_CORES
    )
    xt = nc.dram_tensor("xt", [D, L], BF16, kind="ExternalInput").ap()
    yt = nc.dram_tensor("yt", [D, L], BF16, kind="ExternalInput").ap()
    # wq/wk arrive dt-major: [DT, P, CC*128] (host pre-arranged) so each
    # dt-block is one contiguous 256KB DMA
    wq = nc.dram_tensor("wq", [DT, P, CC * P], BF16, kind="ExternalInput").ap()
    wk = nc.dram_tensor("wk", [DT, P, CC * P], BF16, kind="ExternalInput").ap()
    wv = nc.dram_tensor("wv", [D, DL], BF16, kind="ExternalInput").ap()
    wo = nc.dram_tensor("wo", [DL, U], BF16, kind="ExternalInput").ap()
    out_a = nc.dram_tensor("out_a", [L, U], BF16, kind="ExternalOutput").ap()
    out_c = nc.dram_tensor("out_c", [L, U], BF16, kind="ExternalOutput").ap()

    with tile.TileContext(nc) as tc:
        _mha_body(tc, out_a, out_c, xt, yt, wq, wk, wv, wo)

    nc.compile()
    return nc


def _mha_body(tc, out_a, out_c, xt, yt, wq, wk, wv, wo, dbg=None):
    nc = tc.nc
    from contextlib import ExitStack

    with ExitStack() as ctx:
        persist = ctx.enter_context(tc.tile_pool(name="persist", bufs=1))
        pt_pool = ctx.enter_context(tc.tile_pool(name="pt", bufs=4))
        # ST tiles: [P, 1024] f32 = 2 banks each. Three bufs so the two ST
        # matmuls of a jt-step never wait on the exp drain of the previous
        # step (with 2 bufs the h1 matmul serializes on exp(jt-1,h1), which
        # also kills the row-group concurrency of the K=64 pair).
        ps_wide = ctx.enter_context(tc.tile_pool(name="ps_wide", bufs=3, space="PSUM"))
        # single-bank accumulators (projections, V, ctx, out-proj)
        ps_acc = ctx.enter_context(tc.tile_pool(name="ps_acc", bufs=2, space="PSUM"))
        small = ctx.enter_context(tc.tile_pool(name="small", bufs=4))

        # persistent SBUF tensors
        xt_sb = persist.tile([P, CC, L], BF16, tag="xt")
        yt_sb = persist.tile([P, CC, L], BF16, tag="yt")
        # wq/wk are dt-major (host pre-arranged [DT, P, CC*128]) so the
        # dt0 blocks needed by the first ST land after 0.5MB of weight DMA
        # instead of 2MB
        wq_sb = persist.tile([P, DT, CC * P], BF16, tag="wq")
        wk_sb = persist.tile([P, DT, CC * P], BF16, tag="wk")
        wv_sb = persist.tile([P, CC, DL], BF16, tag="wv")
        wo_sb = persist.tile([P, DT, U], BF16, tag="wo")
        qt_sb = persist.tile([P, DT, L], BF16, tag="qt")
        kt_sb = persist.tile([P, DT, L], BF16, tag="kt")
        # Vaug: per j-chunk, per head a 128-col block; even h: [V_h | ones],
        # odd h: [ones | V_h] (ctx^T lands on the head's own cx partitions)
        va_sb = persist.tile([P, IT, HL * P], BF16, tag="va")
        cx_sb = persist.tile([P, DT, L], BF16, tag="cx")

        # Input DMA rollout. Rails (SP/Pool/ACT) each sustain ~175GB/s with
        # ~256KB-per-config chunks (configs ~0.65us each, transfers rotate
        # across queues). The startup critical path is xt/yt (2MB each, the
        # contraction dim of every projection) plus the dt0 weight blocks;
        # dt-major wq/wk ordering gets the first ST's weights in ~0.5MB.
        #   SP:   xt cc0..7, wv cc4..7
        #   Pool: wq/wk dt0..dt3 interleaved, wv cc0..3, wo
        #   ACT:  yt cc0..7
        wv_r = wv.rearrange("(cc p) d -> p cc d", p=P)
        xt_r = xt.rearrange("(cc p) i -> p cc i", p=P)
        yt_r = yt.rearrange("(cc p) i -> p cc i", p=P)
        for dt in range(DT):
            nc.gpsimd.dma_start(out=wq_sb[:, dt], in_=wq[dt])
            nc.gpsimd.dma_start(out=wk_sb[:, dt], in_=wk[dt])
        for cc in range(CC):
            nc.sync.dma_start(out=xt_sb[:, cc], in_=xt_r[:, cc])
            nc.scalar.dma_start(out=yt_sb[:, cc], in_=yt_r[:, cc])
        for cc in range(4):
            nc.gpsimd.dma_start(out=wv_sb[:, cc], in_=wv_r[:, cc])
            nc.sync.dma_start(out=wv_sb[:, cc + 4], in_=wv_r[:, cc + 4])
        nc.gpsimd.dma_start(out=wo_sb[:], in_=wo.rearrange("(dt p) o -> p dt o", p=P))

        # ones-blocks of Vaug: columns [64,192) mod 256 of each j-chunk
        # (even heads keep V in the low half, odd heads in the high half).
        # One strided memset over half the tensor; the V halves are written
        # by the v_chain drains.
        va_ones = va_sb.rearrange("p it (q s) -> p it q s", s=2 * P)
        nc.vector.memset(va_ones[:, :, :, DH : DH + P], 1.0)

        scale = DH**-0.5

        # ---- chain emitters (each a short burst of independent PE work) ----

        def proj_chain(w_sb, t_sb, rhs_sb, dt, ic, copy_eng="vector"):
            ps = ps_acc.tile([P, NI], F32, tag="acc")
            for cc in range(CC):
                nc.tensor.matmul(
                    ps[:],
                    w_sb[:, dt, cc * P : (cc + 1) * P],
                    rhs_sb[:, cc, ic * NI : (ic + 1) * NI],
                    start=(cc == 0),
                    stop=(cc == CC - 1),
                )
            dst = t_sb[:, dt, ic * NI : (ic + 1) * NI]
            if copy_eng == "vector":
                nc.vector.tensor_copy(dst, ps[:])
            else:
                nc.scalar.copy(dst, ps[:])

        def v_chain(jt):
            ps = ps_acc.tile([P, NI], F32, tag="acc")
            for cc in range(CC):
                nc.tensor.matmul(
                    ps[:],
                    yt_sb[:, cc, jt * P : (jt + 1) * P],
                    wv_sb[:, cc, :],
                    start=(cc == 0),
                    stop=(cc == CC - 1),
                )
            va_blk = va_sb[:, jt].rearrange("p (h s) -> p h s", s=P)
            ps_blk = ps.rearrange("p (h s) -> p h s", s=DH)
            nc.vector.tensor_copy(va_blk[:, 0::2, 0:DH], ps_blk[:, 0::2, :])
            nc.vector.tensor_copy(va_blk[:, 1::2, DH:P], ps_blk[:, 1::2, :])

        # Deferred finishers: the normalize crosses engines (DVE -> gpsimd
        # partition_broadcast -> DVE); emitting the post-broadcast DVE ops
        # immediately would stall the in-order DVE stream (and the PSUM-
        # releasing copies queued behind it) on the gpsimd semaphore.
        # Instead each ctx chain queues them and the next fill slot flushes.
        deferred = []

        def flush_deferred():
            while deferred:
                deferred.pop(0)()

        def ctx_chain(h, ptile, ic, ct=None):
            dt, r0 = divmod(h * DH, P)
            if ct is None:
                ct = ps_acc.tile([P, NI], F32, tag="acc")
                cts = ct[:]
            else:
                cts = ct
            for jt in range(IT):
                nc.tensor.matmul(
                    cts,
                    va_sb[:, jt, h * P : (h + 1) * P],
                    ptile[:, jt, ic * NI : (ic + 1) * NI],
                    start=(jt == 0),
                    stop=(jt == IT - 1),
                )
            # The 64 rowsum rows of ct are identical copies (each ones-column
            # of Vaug reproduces the row sum), so a gpsimd partition
            # broadcast of a single row moves the rowsum to the partitions
            # the ctx rows live on — no DMA round trip. The custom DVE
            # reciprocal only works at base partition 0.
            rc = small.tile([P, NI], F32, tag="rc")
            if r0 == 0:
                # ctx in rows 0:DH, rowsum copies in rows DH:P. The gpsimd
                # broadcast source must sit at partition 0 (Q7 core 0 owns
                # partitions 0:16 and does the read), so this orientation
                # has to move the rowsum down with a SBUF->SBUF DMA.
                rs = small.tile([P, NI], F32, tag="rs")
                nc.vector.tensor_copy(rs[DH:P, :], cts[DH:P, :])
                nc.gpsimd.dma_start(out=rs[0:DH, :], in_=rs[DH:P, :])

                def fin():
                    nc.vector.reciprocal_approx_fast(rc[0:DH, :], rs[0:DH, :])
                    nc.vector.tensor_mul(
                        cx_sb[0:DH, dt, ic * NI : (ic + 1) * NI],
                        cts[0:DH, :],
                        rc[0:DH, :],
                    )
            else:
                # rowsum copies in rows 0:DH, ctx in rows DH:P: reciprocal
                # of a single row at base 0 (all DH rowsum rows are
                # identical), then gpsimd partition-broadcast (the Q7 impl
                # reads the source on core 0 and write-masks partitions
                # [0, channels) absolutely, so broadcast all 128 rows).
                nc.vector.reciprocal_approx_fast(rc[0:1, :], cts[0:1, :])
                nc.gpsimd.partition_broadcast(rc[0:P, :], rc[0:1, :])

                def fin():
                    nc.vector.tensor_mul(
                        cx_sb[DH:P, dt, ic * NI : (ic + 1) * NI],
                        cts[DH:P, :],
                        rc[DH:P, :],
                    )

            deferred.append(fin)

        def po_chain(it, oc, dts, out_ap, copy_eng="vector", po=None, dma_eng=None):
            # out-projection partial over the given d-tiles
            if po is None:
                po = ps_acc.tile([P, NI], F32, tag="acc")
            for k, dt in enumerate(dts):
                nc.tensor.matmul(
                    po[:],
                    cx_sb[:, dt, it * P : (it + 1) * P],
                    wo_sb[:, dt, oc * NI : (oc + 1) * NI],
                    start=(k == 0),
                    stop=(k == len(dts) - 1),
                )
            o_st = small.tile([P, NI], BF16, tag="ost")
            if copy_eng == "vector":
                nc.vector.tensor_copy(o_st[:], po[:])
            else:
                # scalar engine is idle once the exp stream has drained
                nc.scalar.copy(o_st[:], po[:])
            out_r = out_ap.rearrange("(it p) o -> it p o", p=P)
            dma_eng = dma_eng or nc.sync
            dma_eng.dma_start(
                out=out_r[it, :, oc * NI : (oc + 1) * NI], in_=o_st[:]
            )

        # ---- ST + exp for a head pair, fill chains between steps ----

        def st_pair(hp, fills):
            dt = hp
            ptiles = []
            for h_off in range(2):
                pt_tile = pt_pool.tile([P, IT, L], BF16, tag="pt")
                ptiles.append(pt_tile)
            fills = list(fills)
            for jt in range(IT):
                sts = [
                    ps_wide.tile([P, 2 * NI], F32, tag="wide", name=f"st{h}")
                    for h in range(2)
                ]
                # ic-outer / head-inner: the two heads' K=64 matmuls sit on
                # disjoint PE row-groups (partitions 0:64 vs 64:128), so
                # back-to-back issue lets them stream concurrently in the
                # array (tile_position auto-derives from base partitions).
                for ic in range(NIC):
                    for h_off in range(2):
                        r0 = DH * h_off
                        nc.tensor.matmul(
                            sts[h_off][:, ic * NI : (ic + 1) * NI],
                            kt_sb[r0 : r0 + DH, dt, jt * P : (jt + 1) * P],
                            qt_sb[r0 : r0 + DH, dt, ic * NI : (ic + 1) * NI],
                            start=True,
                            stop=True,
                        )
                for h_off in range(2):
                    nc.scalar.activation(
                        ptiles[h_off][:, jt, :],
                        sts[h_off][:],
                        mybir.ActivationFunctionType.Exp,
                        scale=scale,
                    )
                if jt < len(fills):
                    # pending finishers BEFORE this slot's fills: a fill
                    # chain re-claiming the acc tile a pending normalize
                    # still reads would otherwise deadlock the in-order DVE
                    # stream behind the fill's own PSUM-releasing copy
                    # (2-buf acc pool). The one-slot deferral already gave
                    # the gpsimd hop its slack.
                    pending = list(deferred)
                    deferred.clear()
                    for f in pending:
                        f()
                    for f in fills[jt]:
                        f()
            return ptiles

        # ---- schedule ----
        mk = lambda f, *a: (lambda: f(*a))

        # Prologue: all four dt0 chains (QT ic0/ic1, KT ic0/ic1) accumulate
        # per-cc in lockstep, so every xt/yt chunk is consumed the moment it
        # lands and the prologue ends right after the last input chunk —
        # instead of running four serial chains after the data arrived.
        # Four live accumulators: both acc bufs + the two halves of a wide
        # tile (ST pool is idle this early). Drains on the scalar engine
        # (idle until the first exp).
        pq0 = ps_acc.tile([P, NI], F32, tag="acc", name="pq0")
        pq1 = ps_acc.tile([P, NI], F32, tag="acc", name="pq1")
        pkw = ps_wide.tile([P, 2 * NI], F32, tag="wide", name="pkw")
        for cc in range(CC):
            st0 = cc == 0
            sp1 = cc == CC - 1
            nc.tensor.matmul(
                pq0[:], wq_sb[:, 0, cc * P : (cc + 1) * P],
                xt_sb[:, cc, 0:NI], start=st0, stop=sp1,
            )
            nc.tensor.matmul(
                pq1[:], wq_sb[:, 0, cc * P : (cc + 1) * P],
                xt_sb[:, cc, NI : 2 * NI], start=st0, stop=sp1,
            )
            nc.tensor.matmul(
                pkw[:, 0:NI], wk_sb[:, 0, cc * P : (cc + 1) * P],
                yt_sb[:, cc, 0:NI], start=st0, stop=sp1,
            )
            nc.tensor.matmul(
                pkw[:, NI : 2 * NI], wk_sb[:, 0, cc * P : (cc + 1) * P],
                yt_sb[:, cc, NI : 2 * NI], start=st0, stop=sp1,
            )
        nc.scalar.copy(qt_sb[:, 0, 0:NI], pq0[:])
        nc.scalar.copy(qt_sb[:, 0, NI : 2 * NI], pq1[:])
        nc.scalar.copy(kt_sb[:, 0, 0:NI], pkw[:, 0:NI])
        nc.scalar.copy(kt_sb[:, 0, NI : 2 * NI], pkw[:, NI : 2 * NI])

        # pair 0: fill with the 8 V chains
        pt0 = st_pair(0, [[mk(v_chain, jt)] for jt in range(IT)])

        if dbg is not None:
            nc.sync.dma_start(out=dbg[3][0], in_=pt0[0][:])
            nc.sync.dma_start(out=dbg[3][1], in_=pt0[1][:])

        # QT1/KT1 ahead of pair 1 (also covers pair-0 exp tail)
        for ic in range(NIC):
            proj_chain(wq_sb, qt_sb, xt_sb, 1, ic)
        for ic in range(NIC):
            proj_chain(wk_sb, kt_sb, yt_sb, 1, ic)

        # pair 1: fill with ctx of heads 0/1 and QT2/KT2, interleaved so
        # ctx-chain PSUM tiles (whose release waits on the cross-engine
        # normalize) never claim more than every other acc slot
        pt1 = st_pair(
            1,
            [
                [mk(ctx_chain, 0, pt0[0], 0)],
                [mk(proj_chain, wq_sb, qt_sb, xt_sb, 2, 0)],
                [mk(ctx_chain, 0, pt0[0], 1)],
                [mk(proj_chain, wq_sb, qt_sb, xt_sb, 2, 1)],
                [mk(ctx_chain, 1, pt0[1], 0)],
                [mk(proj_chain, wk_sb, kt_sb, yt_sb, 2, 0)],
                [mk(ctx_chain, 1, pt0[1], 1)],
                [mk(proj_chain, wk_sb, kt_sb, yt_sb, 2, 1)],
            ],
        )

        # pair 2: fill with ctx of heads 2/3 and QT3/KT3
        pt2 = st_pair(
            2,
            [
                [mk(ctx_chain, 2, pt1[0], 0)],
                [mk(proj_chain, wq_sb, qt_sb, xt_sb, 3, 0)],
                [mk(ctx_chain, 2, pt1[0], 1)],
                [mk(proj_chain, wq_sb, qt_sb, xt_sb, 3, 1)],
                [mk(ctx_chain, 3, pt1[1], 0)],
                [mk(proj_chain, wk_sb, kt_sb, yt_sb, 3, 0)],
                [mk(ctx_chain, 3, pt1[1], 1)],
                [mk(proj_chain, wk_sb, kt_sb, yt_sb, 3, 1)],
            ],
        )

        # pair 3: fill with ctx of heads 4/5, then out-proj partial A over
        # dt 0..2 (heads 0..5 — heads 4/5 finish in this phase's early slots,
        # so the late slots can already drain 3/4 of the out-projection)
        # out DMAs alternate SP/Pool rails: a single rail's ~0.65us-per-
        # config serialization otherwise backlogs the kernel tail
        poA = [
            mk(
                po_chain, it, oc, (0, 1, 2), out_a, "vector", None,
                (nc.sync, nc.gpsimd)[(2 * it + oc) % 2],
            )
            for it in range(IT)
            for oc in range(NIC)
        ]
        # slot 7 keeps only a small fill: anything more delays the critical
        # tail ctx chains (the exp it must cover is only ~1.3us); the two
        # remaining poA chains instead plug the tail's own idle points
        pt3 = st_pair(
            3,
            [
                [mk(ctx_chain, 4, pt2[0], 0)],
                [mk(ctx_chain, 4, pt2[0], 1)],
                [mk(ctx_chain, 5, pt2[1], 0)],
                [mk(ctx_chain, 5, pt2[1], 1)],
                poA[0:4],
                poA[4:8],
                poA[8:12],
                poA[12:14],
            ],
        )

        # tail: ctx of heads 6/7 on halves of the (by now idle) wide ST
        # tiles — the 2-buf acc pool can't hold four chains through their
        # cross-engine normalizes. ic0 pair first (their finishers gate the
        # first out-proj chains); h6 (DMA normalize) before h7 (broadcast).
        # Only the two READY finishers flush before ctx7ic1 — h6ic1's mul
        # still waits its rs DMA round trip and would stall the in-order
        # DVE stream ahead of the poA copies.
        wtl0 = ps_wide.tile([P, 2 * NI], F32, tag="wide", name="wtl0")
        ctx_chain(6, pt3[0], 0, ct=wtl0[:, 0:NI])
        ctx_chain(7, pt3[1], 0, ct=wtl0[:, NI : 2 * NI])
        wtl1 = ps_wide.tile([P, 2 * NI], F32, tag="wide", name="wtl1")
        ctx_chain(6, pt3[0], 1, ct=wtl1[:, 0:NI])
        deferred.pop(0)()  # fin h6ic0 (rs round trip long done)
        deferred.pop(0)()  # fin h7ic0
        ctx_chain(7, pt3[1], 1, ct=wtl1[:, NI : 2 * NI])
        # the last two out_a chains fill the window where the DVE finishers
        # for the ic1 half are still completing (acc bufs are free: no
        # pending fins target them now)
        for f in poA[14:16]:
            f()
        flush_deferred()  # fins for h6ic1 and h7ic1

        def po_tile_gen():
            # [wide, wide, acc, acc] repeating: the first acc slots the po
            # chains reuse are the ic0 ctx tiles (normalized early); the ic1
            # ctx tiles only come up for reuse once their finishers have
            # run. Wide tiles are used whole (half-sharing serializes on the
            # tile-granular write-after-read hazard).
            while True:
                pw = ps_wide.tile([P, 2 * NI], F32, tag="wide", name="po_w")
                yield pw[:, 0:NI]
                pw = ps_wide.tile([P, 2 * NI], F32, tag="wide", name="po_w")
                yield pw[:, 0:NI]
                yield ps_acc.tile([P, NI], F32, tag="acc", name="po_a")
                yield ps_acc.tile([P, NI], F32, tag="acc", name="po_a")

        po_tiles = po_tile_gen()
        # it-blocks 0..3 only read the ic0-half of cx dt3, whose normalizes
        # are already flushed — emit them before the last (ic1) finishers
        # so their DMA round-trips hide behind real work
        for it in range(IT // 2):
            for oc in range(NIC):
                po_chain(it, oc, (3,), out_c,
                         copy_eng=("scalar", "vector")[oc], po=next(po_tiles),
                         dma_eng=(nc.sync, nc.gpsimd)[oc])
        for it in range(IT // 2, IT):
            for oc in range(NIC):
                po_chain(it, oc, (3,), out_c,
                         copy_eng=("scalar", "vector")[oc], po=next(po_tiles),
                         dma_eng=(nc.sync, nc.gpsimd)[oc])

        if dbg is not None:
            nc.sync.dma_start(out=dbg[0][:], in_=qt_sb[:])
            nc.sync.dma_start(out=dbg[1][:], in_=kt_sb[:])
            nc.sync.dma_start(out=dbg[2][:], in_=va_sb[:])
            nc.sync.dma_start(out=dbg[4][:], in_=cx_sb[:])


_NC_CACHE = None


def _get_nc():
    global _NC_CACHE
    if _NC_CACHE is None:
        _NC_CACHE = _build_kernel()
    return _NC_CACHE


def kernel(x, y, Wq, Wk, Wv, Wo, _trace=False):
    bf = ml_dtypes.bfloat16
    x = np.asarray(x, np.float32)
    y = np.asarray(y, np.float32)
    xtb = [np.ascontiguousarray(np.asarray(x[b]).T).astype(bf) for b in range(B)]
    ytb = [np.ascontiguousarray(np.asarray(y[b]).T).astype(bf) for b in range(B)]
    def _dt_major(w, t):
        # [D, DL] slice -> [DT, P, CC*128]: element (dt, p, cc*128+d) =
        # w[cc*128+p, t*DL + dt*128 + d]  (proj lhsT chunks [P, 128] per
        # (dt, cc), partition dim = contraction rows)
        ws = np.asarray(w)[:, t * DL : (t + 1) * DL]          # [1024, 512]
        ws = ws.reshape(CC, P, DT, P).transpose(2, 1, 0, 3)    # [DT,P,CC,128]
        return np.ascontiguousarray(ws.reshape(DT, P, CC * P)).astype(bf)

    wqs = [_dt_major(Wq, t) for t in range(TP)]
    wks = [_dt_major(Wk, t) for t in range(TP)]
    wvs = [np.ascontiguousarray(np.asarray(Wv)[:, t * DL : (t + 1) * DL]).astype(bf) for t in range(TP)]
    wos = [np.ascontiguousarray(np.asarray(Wo)[t * DL : (t + 1) * DL, :]).astype(bf) for t in range(TP)]

    in_maps = []
    for b in range(B):
        for t in range(TP):
            in_maps.append(
                {
                    "xt": xtb[b],
                    "yt": ytb[b],
                    "wq": wqs[t],
                    "wk": wks[t],
                    "wv": wvs[t],
                    "wo": wos[t],
                }
            )

    nc = _get_nc()
    res = run_bass_kernel_spmd(
        nc, in_maps, core_ids=list(range(N_CORES)), trace=_trace
    )
    out = np.empty((B, L, U), np.float32)
    for b in range(B):
        out[b] = (
            np.asarray(res.results[2 * b]["out_a"], np.float32)
            + np.asarray(res.results[2 * b]["out_c"], np.float32)
            + np.asarray(res.results[2 * b + 1]["out_a"], np.float32)
            + np.asarray(res.results[2 * b + 1]["out_c"], np.float32)
        )
    if _trace:
        return out, res
    return out



# revision 24
# speedup vs baseline: 1.0877x; 1.0852x over previous
"""Multi-head attention (B=4, L=1024, D=1024, H=16, DH=64) on 8 TRN2 NeuronCores.

Sharding: data-parallel over batch (4) x tensor-parallel over heads (2).
Core c = 2*b + t computes, for batch b, heads [t*8, (t+1)*8):
    QT = Wq_t^T X^T, KT = Wk_t^T X^T, V = Y Wv_t        (all bf16 matmuls)
    per head: S^T = K_h Q_h^T; P^T = exp(S^T/8);
              [ctx^T; rowsum] = Vaug_h^T P^T;  ctxn = ctx / rowsum
    O_partial = ctxn^T Wo_t                              (f32, two dt-halves)
Host pre-transposes X/Y, casts to bf16, and sums the four f32 partials
(2 tensor-parallel cores x 2 dt-halves) per batch.

Engines execute their compiled instruction streams in order, so the emission
order is a hand-software-pipelined schedule: every ST (scores) step, whose exp
drain on the scalar engine is slower than the matmuls, is followed by an
independent fill chain (V projection, next d-tile QT/KT, an earlier head's
ctx, or an out-projection partial) so the tensor engine never waits for the
scalar engine to free an ST PSUM tile.

Perf notes (vs the first working version):
  - Input DMA configs are spread across four sequencers (SP/Pool/DVE/ACT);
    a single SP rail configures queues at ~0.6us each, serializing the
    input rollout and starving the PE for the first ~15us.
  - The ones-blocks of Vaug are memset with one strided op (half the data).
  - The first QT/KT drains go to the scalar engine (idle before the exps).
  - Tail: ctx tiles for the last head pair live in the (by then idle) wide
    ST PSUM pool so the out-projection chains get the full 4-slot acc pool;
    tail drains alternate scalar/vector; chain order puts both ic0 ctx
    chains first so their normalize DMA round-trips hide under ic1's PE
    work. Keeping the PE stream dense also holds it at the 2.4GHz p-state
    (it drops to 1.2GHz within ~100ns of going idle).
"""

import numpy as np
import ml_dtypes

import concourse.tile as tile
import concourse.mybir as mybir
from concourse import bacc
from concourse.bass_utils import run_bass_kernel_spmd

B, L, D, U, H = 4, 1024, 1024, 1024, 16
DH = U // H          # 64 head dim
TP = 2               # tensor-parallel ways (heads)
DL = U // TP         # 512 local units
HL = H // TP         # 8 local heads
P = 128              # partitions
NI = 512             # matmul free-dim chunk (one PSUM bank of f32)
CC = D // P          # 8 contraction chunks for projections
DT = DL // P         # 4 local d-tiles
IT = L // P          # 8 i/j tiles
NIC = L // NI        # 2 free chunks of 512
N_CORES = 8

BF16 = mybir.dt.bfloat16
F32 = mybir.dt.float32


def _build_kernel():
    nc = bacc.Bacc(
        "TRN2", target_bir_lowering=False, debug=False, num_devices=N_CORES
    )
    xt = nc.dram_tensor("xt", [D, L], BF16, kind="ExternalInput").ap()
    yt = nc.dram_tensor("yt", [D, L], BF16, kind="ExternalInput").ap()
    # wq/wk arrive dt-major: [DT, P, CC*128] (host pre-arranged) so each
    # dt-block is one contiguous 256KB DMA
    wq = nc.dram_tensor("wq", [DT, P, CC * P], BF16, kind="ExternalInput").ap()
    wk = nc.dram_tensor("wk", [DT, P, CC * P], BF16, kind="ExternalInput").ap()
    wv = nc.dram_tensor("wv", [D, DL], BF16, kind="ExternalInput").ap()
    wo = nc.dram_tensor("wo", [DL, U], BF16, kind="ExternalInput").ap()
    out_a = nc.dram_tensor("out_a", [L, U], BF16, kind="ExternalOutput").ap()
    out_c = nc.dram_tensor("out_c", [L, U], BF16, kind="ExternalOutput").ap()

    with tile.TileContext(nc) as tc:
        _mha_body(tc, out_a, out_c, xt, yt, wq, wk, wv, wo)

    nc.compile()
    return nc


def _mha_body(tc, out_a, out_c, xt, yt, wq, wk, wv, wo, dbg=None):
    nc = tc.nc
    from contextlib import ExitStack

    with ExitStack() as ctx:
        persist = ctx.enter_context(tc.tile_pool(name="persist", bufs=1))
        # P^T tiles are per-pair now; live set = current pair + previous
        # (whose ctx chains consume it)
        pt_pool = ctx.enter_context(tc.tile_pool(name="pt", bufs=2))
        # ST tiles: [P, 1024] f32 = 2 banks each; one per (jt, ic) step
        # holding BOTH heads' 512-blocks, so the two K=64 matmuls land in
        # different banks and stream concurrently on disjoint PE row groups
        ps_wide = ctx.enter_context(tc.tile_pool(name="ps_wide", bufs=2, space="PSUM"))
        # single-bank accumulators (projections, V, ctx, out-proj)
        ps_acc = ctx.enter_context(tc.tile_pool(name="ps_acc", bufs=4, space="PSUM"))
        small = ctx.enter_context(tc.tile_pool(name="small", bufs=4))

        # persistent SBUF tensors
        xt_sb = persist.tile([P, CC, L], BF16, tag="xt")
        yt_sb = persist.tile([P, CC, L], BF16, tag="yt")
        # wq/wk are dt-major (host pre-arranged [DT, P, CC*128]) so the
        # dt0 blocks needed by the first ST land after 0.5MB of weight DMA
        # instead of 2MB
        wq_sb = persist.tile([P, DT, CC * P], BF16, tag="wq")
        wk_sb = persist.tile([P, DT, CC * P], BF16, tag="wk")
        wv_sb = persist.tile([P, CC, DL], BF16, tag="wv")
        wo_sb = persist.tile([P, DT, U], BF16, tag="wo")
        qt_sb = persist.tile([P, DT, L], BF16, tag="qt")
        kt_sb = persist.tile([P, DT, L], BF16, tag="kt")
        # Vaug: per j-chunk, per head a 128-col block; even h: [V_h | ones],
        # odd h: [ones | V_h] (ctx^T lands on the head's own cx partitions)
        va_sb = persist.tile([P, IT, HL * P], BF16, tag="va")
        cx_sb = persist.tile([P, DT, L], BF16, tag="cx")

        # Input DMA rollout. The 16 DMA queues saturate at ~333GB/s
        # aggregate, so the 8MB of inputs take ~24us to land no matter how
        # configs are spread. What matters is that the critical 6MB
        # (xt/yt: the contraction dim of every projection, plus wq/wk)
        # isn't diluted by wv/wo — those 2MB are issued later, on the
        # scalar rail BEHIND the data-dependent prologue copies, so their
        # transfers can't start until the critical set has landed.
        #   SP:   xt cc0..7            (2MB)
        #   Pool: wq/wk dt0..dt3      (2MB, dt-major: dt0 lands in 0.5MB)
        #   ACT:  yt cc0..7            (2MB), then [prologue copies], wv, wo
        wv_r = wv.rearrange("(cc p) d -> p cc d", p=P)
        xt_r = xt.rearrange("(cc p) i -> p cc i", p=P)
        yt_r = yt.rearrange("(cc p) i -> p cc i", p=P)
        for dt in range(DT):
            nc.gpsimd.dma_start(out=wq_sb[:, dt], in_=wq[dt])
            nc.gpsimd.dma_start(out=wk_sb[:, dt], in_=wk[dt])
        for cc in range(CC):
            nc.sync.dma_start(out=xt_sb[:, cc], in_=xt_r[:, cc])
            nc.scalar.dma_start(out=yt_sb[:, cc], in_=yt_r[:, cc])

        # ones-blocks of Vaug: columns [64,192) mod 256 of each j-chunk
        # (even heads keep V in the low half, odd heads in the high half).
        # One strided memset over half the tensor; the V halves are written
        # by the v_chain drains.
        va_ones = va_sb.rearrange("p it (q s) -> p it q s", s=2 * P)
        nc.vector.memset(va_ones[:, :, :, DH : DH + P], 1.0)

        scale = DH**-0.5

        # ---- chain emitters (each a short burst of independent PE work) ----

        def proj_chain(w_sb, t_sb, rhs_sb, dt, ic, copy_eng="vector"):
            ps = ps_acc.tile([P, NI], F32, tag="acc")
            for cc in range(CC):
                nc.tensor.matmul(
                    ps[:],
                    w_sb[:, dt, cc * P : (cc + 1) * P],
                    rhs_sb[:, cc, ic * NI : (ic + 1) * NI],
                    start=(cc == 0),
                    stop=(cc == CC - 1),
                )
            dst = t_sb[:, dt, ic * NI : (ic + 1) * NI]
            if copy_eng == "vector":
                nc.vector.tensor_copy(dst, ps[:])
            else:
                nc.scalar.copy(dst, ps[:])

        def v_chain(jt):
            ps = ps_acc.tile([P, NI], F32, tag="acc")
            for cc in range(CC):
                nc.tensor.matmul(
                    ps[:],
                    yt_sb[:, cc, jt * P : (jt + 1) * P],
                    wv_sb[:, cc, :],
                    start=(cc == 0),
                    stop=(cc == CC - 1),
                )
            va_blk = va_sb[:, jt].rearrange("p (h s) -> p h s", s=P)
            ps_blk = ps.rearrange("p (h s) -> p h s", s=DH)
            nc.vector.tensor_copy(va_blk[:, 0::2, 0:DH], ps_blk[:, 0::2, :])
            nc.vector.tensor_copy(va_blk[:, 1::2, DH:P], ps_blk[:, 1::2, :])

        # Deferred finishers: the normalize crosses engines (DVE -> gpsimd
        # partition_broadcast -> DVE); emitting the post-broadcast DVE ops
        # immediately would stall the in-order DVE stream (and the PSUM-
        # releasing copies queued behind it) on the gpsimd semaphore.
        # Instead each ctx chain queues them and the next fill slot flushes.
        deferred = []

        def flush_deferred():
            while deferred:
                deferred.pop(0)()

        def ctx_chain(h, ptile, ic, ct=None):
            dt, r0 = divmod(h * DH, P)
            if ct is None:
                ct = ps_acc.tile([P, NI], F32, tag="acc")
                cts = ct[:]
            else:
                cts = ct
            for jt in range(IT):
                nc.tensor.matmul(
                    cts,
                    va_sb[:, jt, h * P : (h + 1) * P],
                    ptile[:, jt, ic, h & 1],
                    start=(jt == 0),
                    stop=(jt == IT - 1),
                )
            # The 64 rowsum rows of ct are identical copies (each ones-column
            # of Vaug reproduces the row sum), so a gpsimd partition
            # broadcast of a single row moves the rowsum to the partitions
            # the ctx rows live on — no DMA round trip. The custom DVE
            # reciprocal only works at base partition 0.
            rc = small.tile([P, NI], F32, tag="rc")
            if r0 == 0:
                # ctx in rows 0:DH, rowsum copies in rows DH:P. The gpsimd
                # broadcast source must sit at partition 0 (Q7 core 0 owns
                # partitions 0:16 and does the read), so this orientation
                # has to move the rowsum down with a SBUF->SBUF DMA.
                rs = small.tile([P, NI], F32, tag="rs")
                nc.vector.tensor_copy(rs[DH:P, :], cts[DH:P, :])
                nc.gpsimd.dma_start(out=rs[0:DH, :], in_=rs[DH:P, :])

                def fin():
                    nc.vector.reciprocal_approx_fast(rc[0:DH, :], rs[0:DH, :])
                    nc.vector.tensor_mul(
                        cx_sb[0:DH, dt, ic * NI : (ic + 1) * NI],
                        cts[0:DH, :],
                        rc[0:DH, :],
                    )
            else:
                # rowsum copies in rows 0:DH, ctx in rows DH:P: reciprocal
                # of a single row at base 0 (all DH rowsum rows are
                # identical), then gpsimd partition-broadcast (the Q7 impl
                # reads the source on core 0 and write-masks partitions
                # [0, channels) absolutely, so broadcast all 128 rows).
                nc.vector.reciprocal_approx_fast(rc[0:1, :], cts[0:1, :])
                nc.gpsimd.partition_broadcast(rc[0:P, :], rc[0:1, :])

                def fin():
                    nc.vector.tensor_mul(
                        cx_sb[DH:P, dt, ic * NI : (ic + 1) * NI],
                        cts[DH:P, :],
                        rc[DH:P, :],
                    )

            deferred.append(fin)

        def po_chain(it, oc, dts, out_ap, copy_eng="vector", po=None, dma_eng=None):
            # out-projection partial over the given d-tiles
            if po is None:
                po = ps_acc.tile([P, NI], F32, tag="acc")
            for k, dt in enumerate(dts):
                nc.tensor.matmul(
                    po[:],
                    cx_sb[:, dt, it * P : (it + 1) * P],
                    wo_sb[:, dt, oc * NI : (oc + 1) * NI],
                    start=(k == 0),
                    stop=(k == len(dts) - 1),
                )
            o_st = small.tile([P, NI], BF16, tag="ost")
            if copy_eng == "vector":
                nc.vector.tensor_copy(o_st[:], po[:])
            else:
                # scalar engine is idle once the exp stream has drained
                nc.scalar.copy(o_st[:], po[:])
            out_r = out_ap.rearrange("(it p) o -> it p o", p=P)
            dma_eng = dma_eng or nc.sync
            dma_eng.dma_start(
                out=out_r[it, :, oc * NI : (oc + 1) * NI], in_=o_st[:]
            )

        # ---- ST + exp for a head pair, fill chains between steps ----

        def st_pair(hp, fills):
            # 16 steps of one wide ST tile each: step (ic, jt) computes both
            # heads' [128, 512] score blocks into the two banks of one wide
            # tile (the K=64 matmuls sit on disjoint PE row-groups AND
            # disjoint PSUM banks, so they stream concurrently), and one exp
            # drains the whole tile into the pair's P^T tensor. One wide
            # tile per step keeps the ST pipeline 2 steps deep on a 2-buf
            # pool, leaving 4 banks for the acc pool.
            dt = hp
            ptp = pt_pool.tile([P, IT, NIC, 2, NI], BF16, tag="pt")
            fills = list(fills)
            s = 0
            for ic in range(NIC):
                for jt in range(IT):
                    stw = ps_wide.tile([P, 2 * NI], F32, tag="wide", name="stw")
                    for h_off in range(2):
                        r0 = DH * h_off
                        nc.tensor.matmul(
                            stw[:, h_off * NI : (h_off + 1) * NI],
                            kt_sb[r0 : r0 + DH, dt, jt * P : (jt + 1) * P],
                            qt_sb[r0 : r0 + DH, dt, ic * NI : (ic + 1) * NI],
                            start=True,
                            stop=True,
                        )
                    nc.scalar.activation(
                        ptp[:, jt, ic],
                        stw[:],
                        mybir.ActivationFunctionType.Exp,
                        scale=scale,
                    )
                    if s < len(fills):
                        # pending finishers BEFORE this slot's fills, so a
                        # fill chain never re-claims an acc tile whose
                        # normalize is still queued behind the fill's own
                        # PSUM-releasing copy in the in-order DVE stream
                        pending = list(deferred)
                        deferred.clear()
                        for f in pending:
                            f()
                        for f in fills[s]:
                            f()
                    s += 1
            return ptp

        # ---- schedule ----
        mk = lambda f, *a: (lambda: f(*a))

        # Prologue: all four dt0 chains (QT ic0/ic1, KT ic0/ic1) accumulate
        # per-cc in lockstep across the four acc bufs, so every xt/yt chunk
        # is consumed the moment it lands and the prologue ends right after
        # the last input chunk — instead of running four serial chains
        # after the data arrived. Drains on the scalar engine (idle until
        # the first exp); the wv/wo DMA configs queue behind these copies.
        pro = [ps_acc.tile([P, NI], F32, tag="acc", name=f"pro{k}") for k in range(4)]
        for cc in range(CC):
            st0 = cc == 0
            sp1 = cc == CC - 1
            for ic in range(NIC):
                nc.tensor.matmul(
                    pro[ic][:], wq_sb[:, 0, cc * P : (cc + 1) * P],
                    xt_sb[:, cc, ic * NI : (ic + 1) * NI], start=st0, stop=sp1,
                )
                nc.tensor.matmul(
                    pro[2 + ic][:], wk_sb[:, 0, cc * P : (cc + 1) * P],
                    yt_sb[:, cc, ic * NI : (ic + 1) * NI], start=st0, stop=sp1,
                )
        for ic in range(NIC):
            nc.scalar.copy(qt_sb[:, 0, ic * NI : (ic + 1) * NI], pro[ic][:])
        for ic in range(NIC):
            nc.scalar.copy(kt_sb[:, 0, ic * NI : (ic + 1) * NI], pro[2 + ic][:])
        # wv/wo ride the scalar rail behind the copies above: their 2MB of
        # transfers start only once the critical 6MB has landed
        for cc in range(CC):
            nc.scalar.dma_start(out=wv_sb[:, cc], in_=wv_r[:, cc])
        nc.scalar.dma_start(out=wo_sb[:], in_=wo.rearrange("(dt p) o -> p dt o", p=P))

        # pair 0: QT1/KT1 early (their weights landed with the critical
        # set), V chains in the back half (wv lands mid-phase). Light slots
        # between chains let the exp stream keep pace with ST production.
        q1a = mk(proj_chain, wq_sb, qt_sb, xt_sb, 1, 0)
        q1b = mk(proj_chain, wq_sb, qt_sb, xt_sb, 1, 1)
        k1a = mk(proj_chain, wk_sb, kt_sb, yt_sb, 1, 0)
        k1b = mk(proj_chain, wk_sb, kt_sb, yt_sb, 1, 1)
        pt0 = st_pair(
            0,
            [[q1a], [], [q1b], [], [k1a], [], [k1b], []]
            + [[mk(v_chain, jt)] for jt in range(IT)],
        )

        # pair 1: ctx of heads 0/1 alternating with QT2/KT2
        pt1 = st_pair(
            1,
            [
                [mk(ctx_chain, 0, pt0, 0)], [],
                [mk(proj_chain, wq_sb, qt_sb, xt_sb, 2, 0)], [],
                [mk(ctx_chain, 0, pt0, 1)], [],
                [mk(proj_chain, wq_sb, qt_sb, xt_sb, 2, 1)], [],
                [mk(ctx_chain, 1, pt0, 0)], [],
                [mk(proj_chain, wk_sb, kt_sb, yt_sb, 2, 0)], [],
                [mk(ctx_chain, 1, pt0, 1)], [],
                [mk(proj_chain, wk_sb, kt_sb, yt_sb, 2, 1)], [],
            ],
        )

        # pair 2: ctx of heads 2/3 alternating with QT3/KT3
        pt2 = st_pair(
            2,
            [
                [mk(ctx_chain, 2, pt1, 0)], [],
                [mk(proj_chain, wq_sb, qt_sb, xt_sb, 3, 0)], [],
                [mk(ctx_chain, 2, pt1, 1)], [],
                [mk(proj_chain, wq_sb, qt_sb, xt_sb, 3, 1)], [],
                [mk(ctx_chain, 3, pt1, 0)], [],
                [mk(proj_chain, wk_sb, kt_sb, yt_sb, 3, 0)], [],
                [mk(ctx_chain, 3, pt1, 1)], [],
                [mk(proj_chain, wk_sb, kt_sb, yt_sb, 3, 1)], [],
            ],
        )

        # pair 3: ctx of heads 4/5 in the first half (their finishers all
        # flush by slot 7), then the out_a chains over dt 0..2 — valid for
        # every it-block once heads 4/5 are normalized.
        # out DMAs alternate SP/Pool rails: a single rail's ~0.65us-per-
        # config serialization otherwise backlogs the kernel tail
        poA = [
            mk(
                po_chain, it, oc, (0, 1, 2), out_a, "vector", None,
                (nc.sync, nc.gpsimd)[(2 * it + oc) % 2],
            )
            for it in range(IT)
            for oc in range(NIC)
        ]
        pt3 = st_pair(
            3,
            [
                [mk(ctx_chain, 4, pt2, 0)], [],
                [mk(ctx_chain, 4, pt2, 1)], [],
                [mk(ctx_chain, 5, pt2, 0)], [],
                [mk(ctx_chain, 5, pt2, 1)], [],
                poA[0:2], poA[2:4], poA[4:6], poA[6:8],
                poA[8:10], poA[10:12], poA[12:13], poA[13:14],
            ],
        )

        # tail: ctx of heads 6/7 — each (head, ic) chain gets its own acc
        # tile. ic0 chains first (their finishers gate the first out-proj
        # chains); h6 (DMA normalize) before h7 (broadcast). Only the two
        # READY finishers flush before ctx7ic1 — h6ic1's mul still waits
        # its rs DMA round trip and would stall the in-order DVE stream.
        ctx_chain(6, pt3, 0)
        ctx_chain(7, pt3, 0)
        ctx_chain(6, pt3, 1)
        deferred.pop(0)()  # fin h6ic0 (rs round trip long done)
        deferred.pop(0)()  # fin h7ic0
        ctx_chain(7, pt3, 1)
        # the last two out_a chains fill the window where the DVE finishers
        # for the ic1 half are still completing
        for f in poA[14:16]:
            f()
        flush_deferred()  # fins for h6ic1 and h7ic1

        def po_tile_gen():
            # [wide, wide, acc, acc] repeating: the first acc slots the po
            # chains reuse are the ic0 ctx tiles (normalized early); the ic1
            # ctx tiles only come up for reuse once their finishers have
            # run. Wide tiles are used whole (half-sharing serializes on the
            # tile-granular write-after-read hazard).
            while True:
                pw = ps_wide.tile([P, 2 * NI], F32, tag="wide", name="po_w")
                yield pw[:, 0:NI]
                pw = ps_wide.tile([P, 2 * NI], F32, tag="wide", name="po_w")
                yield pw[:, 0:NI]
                yield ps_acc.tile([P, NI], F32, tag="acc", name="po_a")
                yield ps_acc.tile([P, NI], F32, tag="acc", name="po_a")

        po_tiles = po_tile_gen()
        # it-blocks 0..3 only read the ic0-half of cx dt3, whose normalizes
        # are already flushed — emit them before the last (ic1) finishers
        # so their DMA round-trips hide behind real work
        for it in range(IT // 2):
            for oc in range(NIC):
                po_chain(it, oc, (3,), out_c,
                         copy_eng=("scalar", "vector")[oc], po=next(po_tiles),
                         dma_eng=(nc.sync, nc.gpsimd)[oc])
        for it in range(IT // 2, IT):
            for oc in range(NIC):
                po_chain(it, oc, (3,), out_c,
                         copy_eng=("scalar", "vector")[oc], po=next(po_tiles),
                         dma_eng=(nc.sync, nc.gpsimd)[oc])

        if dbg is not None:
            nc.sync.dma_start(out=dbg[0][:], in_=qt_sb[:])
            nc.sync.dma_start(out=dbg[1][:], in_=kt_sb[:])
            nc.sync.dma_start(out=dbg[2][:], in_=va_sb[:])
            nc.sync.dma_start(out=dbg[4][:], in_=cx_sb[:])


_NC_CACHE = None


def _get_nc():
    global _NC_CACHE
    if _NC_CACHE is None:
        _NC_CACHE = _build_kernel()
    return _NC_CACHE


def kernel(x, y, Wq, Wk, Wv, Wo, _trace=False):
    bf = ml_dtypes.bfloat16
    x = np.asarray(x, np.float32)
    y = np.asarray(y, np.float32)
    xtb = [np.ascontiguousarray(np.asarray(x[b]).T).astype(bf) for b in range(B)]
    ytb = [np.ascontiguousarray(np.asarray(y[b]).T).astype(bf) for b in range(B)]
    def _dt_major(w, t):
        # [D, DL] slice -> [DT, P, CC*128]: element (dt, p, cc*128+d) =
        # w[cc*128+p, t*DL + dt*128 + d]  (proj lhsT chunks [P, 128] per
        # (dt, cc), partition dim = contraction rows)
        ws = np.asarray(w)[:, t * DL : (t + 1) * DL]          # [1024, 512]
        ws = ws.reshape(CC, P, DT, P).transpose(2, 1, 0, 3)    # [DT,P,CC,128]
        return np.ascontiguousarray(ws.reshape(DT, P, CC * P)).astype(bf)

    wqs = [_dt_major(Wq, t) for t in range(TP)]
    wks = [_dt_major(Wk, t) for t in range(TP)]
    wvs = [np.ascontiguousarray(np.asarray(Wv)[:, t * DL : (t + 1) * DL]).astype(bf) for t in range(TP)]
    wos = [np.ascontiguousarray(np.asarray(Wo)[t * DL : (t + 1) * DL, :]).astype(bf) for t in range(TP)]

    in_maps = []
    for b in range(B):
        for t in range(TP):
            in_maps.append(
                {
                    "xt": xtb[b],
                    "yt": ytb[b],
                    "wq": wqs[t],
                    "wk": wks[t],
                    "wv": wvs[t],
                    "wo": wos[t],
                }
            )

    nc = _get_nc()
    res = run_bass_kernel_spmd(
        nc, in_maps, core_ids=list(range(N_CORES)), trace=_trace
    )
    out = np.empty((B, L, U), np.float32)
    for b in range(B):
        out[b] = (
            np.asarray(res.results[2 * b]["out_a"], np.float32)
            + np.asarray(res.results[2 * b]["out_c"], np.float32)
            + np.asarray(res.results[2 * b + 1]["out_a"], np.float32)
            + np.asarray(res.results[2 * b + 1]["out_c"], np.float32)
        )
    if _trace:
        return out, res
    return out



# revision 28
# speedup vs baseline: 1.1098x; 1.0203x over previous
"""Multi-head attention (B=4, L=1024, D=1024, H=16, DH=64) on 8 TRN2 NeuronCores.

Sharding: data-parallel over batch (4) x tensor-parallel over heads (2).
Core c = 2*b + t computes, for batch b, heads [t*8, (t+1)*8):
    QT = Wq_t^T X^T, KT = Wk_t^T X^T, V = Y Wv_t        (all bf16 matmuls)
    per head: S^T = K_h Q_h^T; P^T = exp(S^T/8);
              [ctx^T; rowsum] = Vaug_h^T P^T;  ctxn = ctx / rowsum
    O_partial = ctxn^T Wo_t                              (f32, two dt-halves)
Host pre-transposes X/Y, casts to bf16, and sums the four f32 partials
(2 tensor-parallel cores x 2 dt-halves) per batch.

Engines execute their compiled instruction streams in order, so the emission
order is a hand-software-pipelined schedule: every ST (scores) step, whose exp
drain on the scalar engine is slower than the matmuls, is followed by an
independent fill chain (V projection, next d-tile QT/KT, an earlier head's
ctx, or an out-projection partial) so the tensor engine never waits for the
scalar engine to free an ST PSUM tile.

Perf notes (vs the first working version):
  - Input DMA configs are spread across four sequencers (SP/Pool/DVE/ACT);
    a single SP rail configures queues at ~0.6us each, serializing the
    input rollout and starving the PE for the first ~15us.
  - The ones-blocks of Vaug are memset with one strided op (half the data).
  - The first QT/KT drains go to the scalar engine (idle before the exps).
  - Tail: ctx tiles for the last head pair live in the (by then idle) wide
    ST PSUM pool so the out-projection chains get the full 4-slot acc pool;
    tail drains alternate scalar/vector; chain order puts both ic0 ctx
    chains first so their normalize DMA round-trips hide under ic1's PE
    work. Keeping the PE stream dense also holds it at the 2.4GHz p-state
    (it drops to 1.2GHz within ~100ns of going idle).
"""

import numpy as np
import ml_dtypes

import concourse.tile as tile
import concourse.mybir as mybir
from concourse import bacc
from concourse.bass_utils import run_bass_kernel_spmd

B, L, D, U, H = 4, 1024, 1024, 1024, 16
DH = U // H          # 64 head dim
TP = 2               # tensor-parallel ways (heads)
DL = U // TP         # 512 local units
HL = H // TP         # 8 local heads
P = 128              # partitions
NI = 512             # matmul free-dim chunk (one PSUM bank of f32)
CC = D // P          # 8 contraction chunks for projections
DT = DL // P         # 4 local d-tiles
IT = L // P          # 8 i/j tiles
NIC = L // NI        # 2 free chunks of 512
N_CORES = 8

BF16 = mybir.dt.bfloat16
F32 = mybir.dt.float32


def _build_kernel():
    nc = bacc.Bacc(
        "TRN2", target_bir_lowering=False, debug=False, num_devices=N_CORES
    )
    xt = nc.dram_tensor("xt", [D, L], BF16, kind="ExternalInput").ap()
    yt = nc.dram_tensor("yt", [D, L], BF16, kind="ExternalInput").ap()
    # wq/wk arrive dt-major: [DT, P, CC*128] (host pre-arranged) so each
    # dt-block is one contiguous 256KB DMA
    wq = nc.dram_tensor("wq", [DT, P, CC * P], BF16, kind="ExternalInput").ap()
    wk = nc.dram_tensor("wk", [DT, P, CC * P], BF16, kind="ExternalInput").ap()
    wv = nc.dram_tensor("wv", [D, DL], BF16, kind="ExternalInput").ap()
    wo = nc.dram_tensor("wo", [DL, U], BF16, kind="ExternalInput").ap()
    out_a = nc.dram_tensor("out_a", [L, U], BF16, kind="ExternalOutput").ap()
    out_c = nc.dram_tensor("out_c", [L, U], BF16, kind="ExternalOutput").ap()

    with tile.TileContext(nc) as tc:
        _mha_body(tc, out_a, out_c, xt, yt, wq, wk, wv, wo)

    nc.compile()
    return nc


def _mha_body(tc, out_a, out_c, xt, yt, wq, wk, wv, wo, dbg=None):
    nc = tc.nc
    from contextlib import ExitStack

    with ExitStack() as ctx:
        persist = ctx.enter_context(tc.tile_pool(name="persist", bufs=1))
        # P^T tiles are per-pair now; live set = current pair + previous
        # (whose ctx chains consume it)
        pt_pool = ctx.enter_context(tc.tile_pool(name="pt", bufs=2))
        # ST tiles: [P, 1024] f32 = 2 banks each; one per (jt, ic) step
        # holding BOTH heads' 512-blocks, so the two K=64 matmuls land in
        # different banks and stream concurrently on disjoint PE row groups
        ps_wide = ctx.enter_context(tc.tile_pool(name="ps_wide", bufs=2, space="PSUM"))
        # single-bank accumulators (projections, V, ctx, out-proj)
        ps_acc = ctx.enter_context(tc.tile_pool(name="ps_acc", bufs=4, space="PSUM"))
        small = ctx.enter_context(tc.tile_pool(name="small", bufs=4))

        # persistent SBUF tensors
        xt_sb = persist.tile([P, CC, L], BF16, tag="xt")
        yt_sb = persist.tile([P, CC, L], BF16, tag="yt")
        # wq/wk are dt-major (host pre-arranged [DT, P, CC*128]) so the
        # dt0 blocks needed by the first ST land after 0.5MB of weight DMA
        # instead of 2MB
        wq_sb = persist.tile([P, DT, CC * P], BF16, tag="wq")
        wk_sb = persist.tile([P, DT, CC * P], BF16, tag="wk")
        wv_sb = persist.tile([P, CC, DL], BF16, tag="wv")
        wo_sb = persist.tile([P, DT, U], BF16, tag="wo")
        qt_sb = persist.tile([P, DT, L], BF16, tag="qt")
        kt_sb = persist.tile([P, DT, L], BF16, tag="kt")
        # Vaug: per j-chunk, per head a 128-col block; even h: [V_h | ones],
        # odd h: [ones | V_h] (ctx^T lands on the head's own cx partitions)
        va_sb = persist.tile([P, IT, HL * P], BF16, tag="va")
        cx_sb = persist.tile([P, DT, L], BF16, tag="cx")

        # Input DMA rollout. The 16 DMA queues saturate at ~333GB/s
        # aggregate, so the 8MB of inputs take ~24us to land no matter how
        # configs are spread. What matters is that the critical 6MB
        # (xt/yt: the contraction dim of every projection, plus wq/wk)
        # isn't diluted by wv/wo — those 2MB are issued later, on the
        # scalar rail BEHIND the data-dependent prologue copies, so their
        # transfers can't start until the critical set has landed.
        #   SP:   xt cc0..7            (2MB)
        #   Pool: wq/wk dt0..dt3      (2MB, dt-major: dt0 lands in 0.5MB)
        #   ACT:  yt cc0..7            (2MB), then [prologue copies], wv, wo
        wv_r = wv.rearrange("(cc p) d -> p cc d", p=P)
        xt_r = xt.rearrange("(cc p) i -> p cc i", p=P)
        yt_r = yt.rearrange("(cc p) i -> p cc i", p=P)
        for dt in range(DT):
            nc.gpsimd.dma_start(out=wq_sb[:, dt], in_=wq[dt])
            nc.gpsimd.dma_start(out=wk_sb[:, dt], in_=wk[dt])
        for cc in range(CC):
            nc.sync.dma_start(out=xt_sb[:, cc], in_=xt_r[:, cc])
            nc.scalar.dma_start(out=yt_sb[:, cc], in_=yt_r[:, cc])

        # ones-blocks of Vaug: columns [64,192) mod 256 of each j-chunk
        # (even heads keep V in the low half, odd heads in the high half).
        # One strided memset over half the tensor; the V halves are written
        # by the v_chain drains.
        va_ones = va_sb.rearrange("p it (q s) -> p it q s", s=2 * P)
        nc.vector.memset(va_ones[:, :, :, DH : DH + P], 1.0)

        scale = DH**-0.5

        # ---- chain emitters (each a short burst of independent PE work) ----

        def proj_chain(w_sb, t_sb, rhs_sb, dt, ic, copy_eng="vector"):
            ps = ps_acc.tile([P, NI], F32, tag="acc")
            for cc in range(CC):
                nc.tensor.matmul(
                    ps[:],
                    w_sb[:, dt, cc * P : (cc + 1) * P],
                    rhs_sb[:, cc, ic * NI : (ic + 1) * NI],
                    start=(cc == 0),
                    stop=(cc == CC - 1),
                )
            dst = t_sb[:, dt, ic * NI : (ic + 1) * NI]
            if copy_eng == "vector":
                nc.vector.tensor_copy(dst, ps[:])
            else:
                nc.scalar.copy(dst, ps[:])

        def v_chain(jt):
            ps = ps_acc.tile([P, NI], F32, tag="acc")
            for cc in range(CC):
                nc.tensor.matmul(
                    ps[:],
                    yt_sb[:, cc, jt * P : (jt + 1) * P],
                    wv_sb[:, cc, :],
                    start=(cc == 0),
                    stop=(cc == CC - 1),
                )
            va_blk = va_sb[:, jt].rearrange("p (h s) -> p h s", s=P)
            ps_blk = ps.rearrange("p (h s) -> p h s", s=DH)
            nc.vector.tensor_copy(va_blk[:, 0::2, 0:DH], ps_blk[:, 0::2, :])
            nc.vector.tensor_copy(va_blk[:, 1::2, DH:P], ps_blk[:, 1::2, :])

        # Deferred finishers: the normalize crosses engines (DVE -> gpsimd
        # partition_broadcast -> DVE); emitting the post-broadcast DVE ops
        # immediately would stall the in-order DVE stream (and the PSUM-
        # releasing copies queued behind it) on the gpsimd semaphore.
        # Instead each ctx chain queues them and the next fill slot flushes.
        deferred = []

        def flush_deferred():
            while deferred:
                deferred.pop(0)()

        def ctx_chain(h, ptile, ic, ct=None):
            if ptile is None:
                # fill inside the pair whose P^T this chain consumes
                ptile = st_pair.current
            dt, r0 = divmod(h * DH, P)
            if ct is None:
                ct = ps_acc.tile([P, NI], F32, tag="acc")
                cts = ct[:]
            else:
                cts = ct
            for jt in range(IT):
                nc.tensor.matmul(
                    cts,
                    va_sb[:, jt, h * P : (h + 1) * P],
                    ptile[:, jt, ic, h & 1],
                    start=(jt == 0),
                    stop=(jt == IT - 1),
                )
            # The 64 rowsum rows of ct are identical copies (each ones-column
            # of Vaug reproduces the row sum), so a gpsimd partition
            # broadcast of a single row moves the rowsum to the partitions
            # the ctx rows live on — no DMA round trip. The custom DVE
            # reciprocal only works at base partition 0.
            rc = small.tile([P, NI], F32, tag="rc")
            if r0 == 0:
                # ctx in rows 0:DH, rowsum copies in rows DH:P. The gpsimd
                # broadcast source must sit at partition 0 (Q7 core 0 owns
                # partitions 0:16 and does the read), so this orientation
                # has to move the rowsum down with a SBUF->SBUF DMA.
                rs = small.tile([P, NI], F32, tag="rs")
                nc.vector.tensor_copy(rs[DH:P, :], cts[DH:P, :])
                nc.gpsimd.dma_start(out=rs[0:DH, :], in_=rs[DH:P, :])

                def fin():
                    nc.vector.reciprocal_approx_fast(rc[0:DH, :], rs[0:DH, :])
                    nc.vector.tensor_mul(
                        cx_sb[0:DH, dt, ic * NI : (ic + 1) * NI],
                        cts[0:DH, :],
                        rc[0:DH, :],
                    )
            else:
                # rowsum copies in rows 0:DH, ctx in rows DH:P: reciprocal
                # of a single row at base 0 (all DH rowsum rows are
                # identical), then gpsimd partition-broadcast (the Q7 impl
                # reads the source on core 0 and write-masks partitions
                # [0, channels) absolutely, so broadcast all 128 rows).
                nc.vector.reciprocal_approx_fast(rc[0:1, :], cts[0:1, :])
                nc.gpsimd.partition_broadcast(rc[0:P, :], rc[0:1, :])

                def fin():
                    nc.vector.tensor_mul(
                        cx_sb[DH:P, dt, ic * NI : (ic + 1) * NI],
                        cts[DH:P, :],
                        rc[DH:P, :],
                    )

            deferred.append(fin)

        def po_chain(it, oc, dts, out_ap, copy_eng="vector", po=None, dma_eng=None):
            # out-projection partial over the given d-tiles
            if po is None:
                po = ps_acc.tile([P, NI], F32, tag="acc")
            for k, dt in enumerate(dts):
                nc.tensor.matmul(
                    po[:],
                    cx_sb[:, dt, it * P : (it + 1) * P],
                    wo_sb[:, dt, oc * NI : (oc + 1) * NI],
                    start=(k == 0),
                    stop=(k == len(dts) - 1),
                )
            o_st = small.tile([P, NI], BF16, tag="ost")
            if copy_eng == "vector":
                nc.vector.tensor_copy(o_st[:], po[:])
            else:
                # scalar engine is idle once the exp stream has drained
                nc.scalar.copy(o_st[:], po[:])
            out_r = out_ap.rearrange("(it p) o -> it p o", p=P)
            dma_eng = dma_eng or nc.sync
            dma_eng.dma_start(
                out=out_r[it, :, oc * NI : (oc + 1) * NI], in_=o_st[:]
            )

        # ---- ST + exp for a head pair, fill chains between steps ----

        def st_pair(hp, fills):
            # 16 steps of one wide ST tile each: step (ic, jt) computes both
            # heads' [128, 512] score blocks into the two banks of one wide
            # tile (the K=64 matmuls sit on disjoint PE row-groups AND
            # disjoint PSUM banks, so they stream concurrently), and one exp
            # drains the whole tile into the pair's P^T tensor. One wide
            # tile per step keeps the ST pipeline 2 steps deep on a 2-buf
            # pool, leaving 4 banks for the acc pool.
            dt = hp
            ptp = pt_pool.tile([P, IT, NIC, 2, NI], BF16, tag="pt")
            st_pair.current = ptp
            fills = list(fills)
            s = 0
            for ic in range(NIC):
                for jt in range(IT):
                    stw = ps_wide.tile([P, 2 * NI], F32, tag="wide", name="stw")
                    for h_off in range(2):
                        r0 = DH * h_off
                        nc.tensor.matmul(
                            stw[:, h_off * NI : (h_off + 1) * NI],
                            kt_sb[r0 : r0 + DH, dt, jt * P : (jt + 1) * P],
                            qt_sb[r0 : r0 + DH, dt, ic * NI : (ic + 1) * NI],
                            start=True,
                            stop=True,
                        )
                    nc.scalar.activation(
                        ptp[:, jt, ic],
                        stw[:],
                        mybir.ActivationFunctionType.Exp,
                        scale=scale,
                    )
                    if s < len(fills):
                        # pending finishers BEFORE this slot's fills, so a
                        # fill chain never re-claims an acc tile whose
                        # normalize is still queued behind the fill's own
                        # PSUM-releasing copy in the in-order DVE stream
                        pending = list(deferred)
                        deferred.clear()
                        for f in pending:
                            f()
                        for f in fills[s]:
                            f()
                    s += 1
            return ptp

        # ---- schedule ----
        mk = lambda f, *a: (lambda: f(*a))

        # Prologue: all four dt0 chains (QT ic0/ic1, KT ic0/ic1) accumulate
        # per-cc in lockstep across the four acc bufs, so every xt/yt chunk
        # is consumed the moment it lands and the prologue ends right after
        # the last input chunk — instead of running four serial chains
        # after the data arrived. Drains on the scalar engine (idle until
        # the first exp); the wv/wo DMA configs queue behind these copies.
        pro = [ps_acc.tile([P, NI], F32, tag="acc", name=f"pro{k}") for k in range(4)]
        for cc in range(CC):
            st0 = cc == 0
            sp1 = cc == CC - 1
            for ic in range(NIC):
                nc.tensor.matmul(
                    pro[ic][:], wq_sb[:, 0, cc * P : (cc + 1) * P],
                    xt_sb[:, cc, ic * NI : (ic + 1) * NI], start=st0, stop=sp1,
                )
                nc.tensor.matmul(
                    pro[2 + ic][:], wk_sb[:, 0, cc * P : (cc + 1) * P],
                    yt_sb[:, cc, ic * NI : (ic + 1) * NI], start=st0, stop=sp1,
                )
        for ic in range(NIC):
            nc.scalar.copy(qt_sb[:, 0, ic * NI : (ic + 1) * NI], pro[ic][:])
        for ic in range(NIC):
            nc.scalar.copy(kt_sb[:, 0, ic * NI : (ic + 1) * NI], pro[2 + ic][:])
        # wv/wo ride the scalar rail behind the copies above: their 2MB of
        # transfers start only once the critical 6MB has landed
        for cc in range(CC):
            nc.scalar.dma_start(out=wv_sb[:, cc], in_=wv_r[:, cc])
        nc.scalar.dma_start(out=wo_sb[:], in_=wo.rearrange("(dt p) o -> p dt o", p=P))

        # pair 0: QT1/KT1 early (their weights landed with the critical
        # set), V chains in the back half (wv lands mid-phase). Light slots
        # between chains let the exp stream keep pace with ST production.
        q1a = mk(proj_chain, wq_sb, qt_sb, xt_sb, 1, 0)
        q1b = mk(proj_chain, wq_sb, qt_sb, xt_sb, 1, 1)
        k1a = mk(proj_chain, wk_sb, kt_sb, yt_sb, 1, 0)
        k1b = mk(proj_chain, wk_sb, kt_sb, yt_sb, 1, 1)
        pt0 = st_pair(
            0,
            [[q1a], [], [q1b], [], [k1a], [], [k1b], []]
            + [[mk(v_chain, jt)] for jt in range(IT)],
        )

        # pair 1: ctx of heads 0/1 alternating with QT2/KT2
        pt1 = st_pair(
            1,
            [
                [mk(ctx_chain, 0, pt0, 0)], [],
                [mk(proj_chain, wq_sb, qt_sb, xt_sb, 2, 0)], [],
                [mk(ctx_chain, 0, pt0, 1)], [],
                [mk(proj_chain, wq_sb, qt_sb, xt_sb, 2, 1)], [],
                [mk(ctx_chain, 1, pt0, 0)], [],
                [mk(proj_chain, wk_sb, kt_sb, yt_sb, 2, 0)], [],
                [mk(ctx_chain, 1, pt0, 1)], [],
                [mk(proj_chain, wk_sb, kt_sb, yt_sb, 2, 1)], [],
            ],
        )

        # pair 2: ctx of heads 2/3 alternating with QT3/KT3
        pt2 = st_pair(
            2,
            [
                [mk(ctx_chain, 2, pt1, 0)], [],
                [mk(proj_chain, wq_sb, qt_sb, xt_sb, 3, 0)], [],
                [mk(ctx_chain, 2, pt1, 1)], [],
                [mk(proj_chain, wq_sb, qt_sb, xt_sb, 3, 1)], [],
                [mk(ctx_chain, 3, pt1, 0)], [],
                [mk(proj_chain, wk_sb, kt_sb, yt_sb, 3, 0)], [],
                [mk(ctx_chain, 3, pt1, 1)], [],
                [mk(proj_chain, wk_sb, kt_sb, yt_sb, 3, 1)], [],
            ],
        )

        # out_a chains over dt 0..2: valid for every it-block once heads
        # 4/5 are normalized (by pair-3 slot 7). Copies alternate
        # scalar/vector (both drain engines have slack mid-phase); DMAs
        # alternate SP/Pool rails so no single rail's ~0.65us-per-config
        # serialization backlogs the kernel tail.
        poA = [
            mk(
                po_chain, it, oc, (0, 1, 2), out_a,
                ("vector", "scalar")[(2 * it + oc) % 2], None,
                (nc.sync, nc.gpsimd)[(2 * it + oc) % 2],
            )
            for it in range(IT)
            for oc in range(NIC)
        ]

        # merged out_c unit for one it-block: both oc halves into one
        # [P, L] staging tile, one 256KB DMA (halves the tail's config
        # count). In-phase units draw two acc tiles; tail units use the
        # two halves of one (by then idle) wide ST tile.
        out_cr = out_c.rearrange("(it p) o -> it p o", p=P)

        def oc_unit(it, p0=None, p1=None):
            if p0 is None:
                p0 = ps_acc.tile([P, NI], F32, tag="acc", name="ocp0")
                p1 = ps_acc.tile([P, NI], F32, tag="acc", name="ocp1")
            for oc, po in ((0, p0), (1, p1)):
                nc.tensor.matmul(
                    po,
                    cx_sb[:, 3, it * P : (it + 1) * P],
                    wo_sb[:, 3, oc * NI : (oc + 1) * NI],
                    start=True,
                    stop=True,
                )
            ot = small.tile([P, L], BF16, tag="ost2", name=f"oc{it}")
            nc.scalar.copy(ot[:, 0:NI], p0)
            nc.vector.tensor_copy(ot[:, NI : 2 * NI], p1)
            (nc.sync, nc.gpsimd)[it % 2].dma_start(out=out_cr[it], in_=ot[:])

        # pair 3: ctx of heads 4/5 first (finishers flushed by slot 7),
        # then — because the ic-outer step order finishes all ic0 exps at
        # mid-phase — ctx of heads 6/7 ic0 runs IN-phase (slots 8/9), their
        # finishers flush by slot 11, and the it0..3 out_c units (which
        # only read the ic0 columns of cx dt3) drain in the last slots.
        # Only heads 6/7 ic1 + it4..7 remain for the tail.
        pt3 = st_pair(
            3,
            [
                [mk(ctx_chain, 4, pt2, 0)], [],
                [mk(ctx_chain, 4, pt2, 1)], [],
                [mk(ctx_chain, 5, pt2, 0)], [],
                [mk(ctx_chain, 5, pt2, 1)], [],
                [mk(ctx_chain, 6, None, 0)],
                [mk(ctx_chain, 7, None, 0), poA[0]],
                poA[1:3],
                poA[3:5],
                poA[5:7] + [mk(oc_unit, 0)],
                poA[7:9] + [mk(oc_unit, 1)],
                poA[9:11] + [mk(oc_unit, 2)],
                poA[11:14] + [mk(oc_unit, 3)],
            ],
        )

        # tail: ctx of heads 6/7 ic1, the last two out_a chains plugging
        # the normalize-latency window, then the it4..7 out_c units on
        # wide-tile halves.
        ctx_chain(6, pt3, 1)
        ctx_chain(7, pt3, 1)
        for f in poA[14:16]:
            f()
        flush_deferred()  # fins for h6ic1 and h7ic1
        for it in range(IT // 2, IT):
            pw = ps_wide.tile([P, 2 * NI], F32, tag="wide", name="po_w")
            oc_unit(it, pw[:, 0:NI], pw[:, NI : 2 * NI])

        if dbg is not None:
            nc.sync.dma_start(out=dbg[0][:], in_=qt_sb[:])
            nc.sync.dma_start(out=dbg[1][:], in_=kt_sb[:])
            nc.sync.dma_start(out=dbg[2][:], in_=va_sb[:])
            nc.sync.dma_start(out=dbg[4][:], in_=cx_sb[:])


_NC_CACHE = None


def _get_nc():
    global _NC_CACHE
    if _NC_CACHE is None:
        _NC_CACHE = _build_kernel()
    return _NC_CACHE


def kernel(x, y, Wq, Wk, Wv, Wo, _trace=False):
    bf = ml_dtypes.bfloat16
    x = np.asarray(x, np.float32)
    y = np.asarray(y, np.float32)
    xtb = [np.ascontiguousarray(np.asarray(x[b]).T).astype(bf) for b in range(B)]
    ytb = [np.ascontiguousarray(np.asarray(y[b]).T).astype(bf) for b in range(B)]
    def _dt_major(w, t):
        # [D, DL] slice -> [DT, P, CC*128]: element (dt, p, cc*128+d) =
        # w[cc*128+p, t*DL + dt*128 + d]  (proj lhsT chunks [P, 128] per
        # (dt, cc), partition dim = contraction rows)
        ws = np.asarray(w)[:, t * DL : (t + 1) * DL]          # [1024, 512]
        ws = ws.reshape(CC, P, DT, P).transpose(2, 1, 0, 3)    # [DT,P,CC,128]
        return np.ascontiguousarray(ws.reshape(DT, P, CC * P)).astype(bf)

    wqs = [_dt_major(Wq, t) for t in range(TP)]
    wks = [_dt_major(Wk, t) for t in range(TP)]
    wvs = [np.ascontiguousarray(np.asarray(Wv)[:, t * DL : (t + 1) * DL]).astype(bf) for t in range(TP)]
    wos = [np.ascontiguousarray(np.asarray(Wo)[t * DL : (t + 1) * DL, :]).astype(bf) for t in range(TP)]

    in_maps = []
    for b in range(B):
        for t in range(TP):
            in_maps.append(
                {
                    "xt": xtb[b],
                    "yt": ytb[b],
                    "wq": wqs[t],
                    "wk": wks[t],
                    "wv": wvs[t],
                    "wo": wos[t],
                }
            )

    nc = _get_nc()
    res = run_bass_kernel_spmd(
        nc, in_maps, core_ids=list(range(N_CORES)), trace=_trace
    )
    out = np.empty((B, L, U), np.float32)
    for b in range(B):
        out[b] = (
            np.asarray(res.results[2 * b]["out_a"], np.float32)
            + np.asarray(res.results[2 * b]["out_c"], np.float32)
            + np.asarray(res.results[2 * b + 1]["out_a"], np.float32)
            + np.asarray(res.results[2 * b + 1]["out_c"], np.float32)
        )
    if _trace:
        return out, res
    return out



# revision 32
# speedup vs baseline: 1.1695x; 1.0538x over previous
"""Multi-head attention (B=4, L=1024, D=1024, H=16, DH=64) on 8 TRN2 NeuronCores.

Sharding: data-parallel over batch (4) x tensor-parallel over heads (2).
Core c = 2*b + t computes, for batch b, heads [t*8, (t+1)*8):
    QT = Wq_t^T X^T, KT = Wk_t^T X^T, V = Y Wv_t        (all bf16 matmuls)
    per head: S^T = K_h Q_h^T; P^T = exp(S^T/8);
              [ctx^T; rowsum] = Vaug_h^T P^T;  ctxn = ctx / rowsum
    O_partial = ctxn^T Wo_t                              (f32, two dt-halves)
Host pre-transposes X/Y, casts to bf16, and sums the four f32 partials
(2 tensor-parallel cores x 2 dt-halves) per batch.

Engines execute their compiled instruction streams in order, so the emission
order is a hand-software-pipelined schedule: every ST (scores) step, whose exp
drain on the scalar engine is slower than the matmuls, is followed by an
independent fill chain (V projection, next d-tile QT/KT, an earlier head's
ctx, or an out-projection partial) so the tensor engine never waits for the
scalar engine to free an ST PSUM tile.

Perf notes (vs the first working version):
  - Input DMA configs are spread across four sequencers (SP/Pool/DVE/ACT);
    a single SP rail configures queues at ~0.6us each, serializing the
    input rollout and starving the PE for the first ~15us.
  - The ones-blocks of Vaug are memset with one strided op (half the data).
  - The first QT/KT drains go to the scalar engine (idle before the exps).
  - Tail: ctx tiles for the last head pair live in the (by then idle) wide
    ST PSUM pool so the out-projection chains get the full 4-slot acc pool;
    tail drains alternate scalar/vector; chain order puts both ic0 ctx
    chains first so their normalize DMA round-trips hide under ic1's PE
    work. Keeping the PE stream dense also holds it at the 2.4GHz p-state
    (it drops to 1.2GHz within ~100ns of going idle).
"""

import numpy as np
import ml_dtypes

import concourse.tile as tile
import concourse.mybir as mybir
from concourse import bacc
from concourse.bass_utils import run_bass_kernel_spmd

B, L, D, U, H = 4, 1024, 1024, 1024, 16
DH = U // H          # 64 head dim
TP = 2               # tensor-parallel ways (heads)
DL = U // TP         # 512 local units
HL = H // TP         # 8 local heads
P = 128              # partitions
NI = 512             # matmul free-dim chunk (one PSUM bank of f32)
CC = D // P          # 8 contraction chunks for projections
DT = DL // P         # 4 local d-tiles
IT = L // P          # 8 i/j tiles
NIC = L // NI        # 2 free chunks of 512
N_CORES = 8

BF16 = mybir.dt.bfloat16
F32 = mybir.dt.float32


def _build_kernel():
    nc = bacc.Bacc(
        "TRN2", target_bir_lowering=False, debug=False, num_devices=N_CORES
    )
    xt = nc.dram_tensor("xt", [D, L], BF16, kind="ExternalInput").ap()
    yt = nc.dram_tensor("yt", [D, L], BF16, kind="ExternalInput").ap()
    # wq/wk arrive dt-major: [DT, P, CC*128] (host pre-arranged) so each
    # dt-block is one contiguous 256KB DMA
    wq = nc.dram_tensor("wq", [DT, P, CC * P], BF16, kind="ExternalInput").ap()
    wk = nc.dram_tensor("wk", [DT, P, CC * P], BF16, kind="ExternalInput").ap()
    wv = nc.dram_tensor("wv", [D, DL], BF16, kind="ExternalInput").ap()
    wo = nc.dram_tensor("wo", [DL, U], BF16, kind="ExternalInput").ap()
    out_a = nc.dram_tensor("out_a", [L, U], BF16, kind="ExternalOutput").ap()
    out_c = nc.dram_tensor("out_c", [L, U], BF16, kind="ExternalOutput").ap()

    with tile.TileContext(nc) as tc:
        _mha_body(tc, out_a, out_c, xt, yt, wq, wk, wv, wo)

    nc.compile()
    return nc


def _mha_body(tc, out_a, out_c, xt, yt, wq, wk, wv, wo, dbg=None):
    nc = tc.nc
    from contextlib import ExitStack

    with ExitStack() as ctx:
        persist = ctx.enter_context(tc.tile_pool(name="persist", bufs=1))
        # P^T tiles are per-pair now; live set = current pair + previous
        # (whose ctx chains consume it)
        pt_pool = ctx.enter_context(tc.tile_pool(name="pt", bufs=2))
        # ST tiles: [P, 1024] f32 = 2 banks each; one per (jt, ic) step
        # holding BOTH heads' 512-blocks, so the two K=64 matmuls land in
        # different banks and stream concurrently on disjoint PE row groups
        ps_wide = ctx.enter_context(tc.tile_pool(name="ps_wide", bufs=2, space="PSUM"))
        # single-bank accumulators (projections, V, ctx, out-proj)
        ps_acc = ctx.enter_context(tc.tile_pool(name="ps_acc", bufs=4, space="PSUM"))
        small = ctx.enter_context(tc.tile_pool(name="small", bufs=4))

        # persistent SBUF tensors
        xt_sb = persist.tile([P, CC, L], BF16, tag="xt")
        yt_sb = persist.tile([P, CC, L], BF16, tag="yt")
        # wq/wk are dt-major (host pre-arranged [DT, P, CC*128]) so the
        # dt0 blocks needed by the first ST land after 0.5MB of weight DMA
        # instead of 2MB
        wq_sb = persist.tile([P, DT, CC * P], BF16, tag="wq")
        wk_sb = persist.tile([P, DT, CC * P], BF16, tag="wk")
        wv_sb = persist.tile([P, CC, DL], BF16, tag="wv")
        wo_sb = persist.tile([P, DT, U], BF16, tag="wo")
        qt_sb = persist.tile([P, DT, L], BF16, tag="qt")
        kt_sb = persist.tile([P, DT, L], BF16, tag="kt")
        # Vaug: per j-chunk, per head a 128-col block; even h: [V_h | ones],
        # odd h: [ones | V_h] (ctx^T lands on the head's own cx partitions)
        va_sb = persist.tile([P, IT, HL * P], BF16, tag="va")
        cx_sb = persist.tile([P, DT, L], BF16, tag="cx")

        # Input DMA rollout. The 16 DMA queues saturate at ~333GB/s
        # aggregate, so the 8MB of inputs take ~24us to land no matter how
        # configs are spread. What matters is that the critical 6MB
        # (xt/yt: the contraction dim of every projection, plus wq/wk)
        # isn't diluted by wv/wo — those 2MB are issued later, on the
        # scalar rail BEHIND the data-dependent prologue copies, so their
        # transfers can't start until the critical set has landed.
        #   SP:   xt cc0..7            (2MB)
        #   Pool: wq/wk dt0..dt3      (2MB, dt-major: dt0 lands in 0.5MB)
        #   ACT:  yt cc0..7            (2MB), then [prologue copies], wv, wo
        wv_r = wv.rearrange("(cc p) d -> p cc d", p=P)
        xt_r = xt.rearrange("(cc p) i -> p cc i", p=P)
        yt_r = yt.rearrange("(cc p) i -> p cc i", p=P)
        nc.gpsimd.dma_start(out=wq_sb[:, 0], in_=wq[0])
        nc.gpsimd.dma_start(out=wk_sb[:, 0], in_=wk[0])
        for cc in range(CC):
            nc.sync.dma_start(out=xt_sb[:, cc], in_=xt_r[:, cc])
            nc.scalar.dma_start(out=yt_sb[:, cc], in_=yt_r[:, cc])

        # ones-blocks of Vaug: columns [64,192) mod 256 of each j-chunk
        # (even heads keep V in the low half, odd heads in the high half).
        # One strided memset over half the tensor; the V halves are written
        # by the v_chain drains.
        va_ones = va_sb.rearrange("p it (q s) -> p it q s", s=2 * P)
        nc.vector.memset(va_ones[:, :, :, DH : DH + P], 1.0)

        scale = DH**-0.5

        # ---- chain emitters (each a short burst of independent PE work) ----

        def proj_chain(w_sb, t_sb, rhs_sb, dt, ic, copy_eng="vector"):
            ps = ps_acc.tile([P, NI], F32, tag="acc")
            for cc in range(CC):
                nc.tensor.matmul(
                    ps[:],
                    w_sb[:, dt, cc * P : (cc + 1) * P],
                    rhs_sb[:, cc, ic * NI : (ic + 1) * NI],
                    start=(cc == 0),
                    stop=(cc == CC - 1),
                )
            dst = t_sb[:, dt, ic * NI : (ic + 1) * NI]
            if copy_eng == "vector":
                nc.vector.tensor_copy(dst, ps[:])
            else:
                nc.scalar.copy(dst, ps[:])

        def v_chain(jt):
            ps = ps_acc.tile([P, NI], F32, tag="acc")
            for cc in range(CC):
                nc.tensor.matmul(
                    ps[:],
                    yt_sb[:, cc, jt * P : (jt + 1) * P],
                    wv_sb[:, cc, :],
                    start=(cc == 0),
                    stop=(cc == CC - 1),
                )
            va_blk = va_sb[:, jt].rearrange("p (h s) -> p h s", s=P)
            ps_blk = ps.rearrange("p (h s) -> p h s", s=DH)
            nc.vector.tensor_copy(va_blk[:, 0::2, 0:DH], ps_blk[:, 0::2, :])
            nc.vector.tensor_copy(va_blk[:, 1::2, DH:P], ps_blk[:, 1::2, :])

        # Deferred finishers: the normalize crosses engines (DVE -> gpsimd
        # partition_broadcast -> DVE); emitting the post-broadcast DVE ops
        # immediately would stall the in-order DVE stream (and the PSUM-
        # releasing copies queued behind it) on the gpsimd semaphore.
        # Instead each ctx chain queues them and the next fill slot flushes.
        deferred = []

        def flush_deferred():
            while deferred:
                deferred.pop(0)()

        def ctx_chain(h, ptile, ic, ct=None):
            if ptile is None:
                # fill inside the pair whose P^T this chain consumes
                ptile = st_pair.current
            dt, r0 = divmod(h * DH, P)
            if ct is None:
                ct = ps_acc.tile([P, NI], F32, tag="acc")
                cts = ct[:]
            else:
                cts = ct
            for jt in range(IT):
                nc.tensor.matmul(
                    cts,
                    va_sb[:, jt, h * P : (h + 1) * P],
                    ptile[:, jt, ic, h & 1],
                    start=(jt == 0),
                    stop=(jt == IT - 1),
                )
            # The 64 rowsum rows of ct are identical copies (each ones-column
            # of Vaug reproduces the row sum), so a gpsimd partition
            # broadcast of a single row moves the rowsum to the partitions
            # the ctx rows live on — no DMA round trip. The custom DVE
            # reciprocal only works at base partition 0.
            rc = small.tile([P, NI], F32, tag="rc")
            if r0 == 0:
                # ctx in rows 0:DH, rowsum copies in rows DH:P. The gpsimd
                # broadcast source must sit at partition 0 (Q7 core 0 owns
                # partitions 0:16 and does the read), so this orientation
                # has to move the rowsum down with a SBUF->SBUF DMA.
                rs = small.tile([P, NI], F32, tag="rs")
                nc.vector.tensor_copy(rs[DH:P, :], cts[DH:P, :])
                nc.gpsimd.dma_start(out=rs[0:DH, :], in_=rs[DH:P, :])

                def fin():
                    nc.vector.reciprocal_approx_fast(rc[0:DH, :], rs[0:DH, :])
                    nc.vector.tensor_mul(
                        cx_sb[0:DH, dt, ic * NI : (ic + 1) * NI],
                        cts[0:DH, :],
                        rc[0:DH, :],
                    )
            else:
                # rowsum copies in rows 0:DH, ctx in rows DH:P: reciprocal
                # of a single row at base 0 (all DH rowsum rows are
                # identical), then gpsimd partition-broadcast (the Q7 impl
                # reads the source on core 0 and write-masks partitions
                # [0, channels) absolutely, so broadcast all 128 rows).
                nc.vector.reciprocal_approx_fast(rc[0:1, :], cts[0:1, :])
                nc.gpsimd.partition_broadcast(rc[0:P, :], rc[0:1, :])

                def fin():
                    nc.vector.tensor_mul(
                        cx_sb[DH:P, dt, ic * NI : (ic + 1) * NI],
                        cts[DH:P, :],
                        rc[DH:P, :],
                    )

            deferred.append(fin)

        def po_chain(it, oc, dts, out_ap, copy_eng="vector", po=None, dma_eng=None):
            # out-projection partial over the given d-tiles
            if po is None:
                po = ps_acc.tile([P, NI], F32, tag="acc")
            for k, dt in enumerate(dts):
                nc.tensor.matmul(
                    po[:],
                    cx_sb[:, dt, it * P : (it + 1) * P],
                    wo_sb[:, dt, oc * NI : (oc + 1) * NI],
                    start=(k == 0),
                    stop=(k == len(dts) - 1),
                )
            o_st = small.tile([P, NI], BF16, tag="ost")
            if copy_eng == "vector":
                nc.vector.tensor_copy(o_st[:], po[:])
            else:
                # scalar engine is idle once the exp stream has drained
                nc.scalar.copy(o_st[:], po[:])
            out_r = out_ap.rearrange("(it p) o -> it p o", p=P)
            dma_eng = dma_eng or nc.sync
            dma_eng.dma_start(
                out=out_r[it, :, oc * NI : (oc + 1) * NI], in_=o_st[:]
            )

        # ---- ST + exp for a head pair, fill chains between steps ----

        def st_pair(hp, fills):
            # 16 steps of one wide ST tile each: step (ic, jt) computes both
            # heads' [128, 512] score blocks into the two banks of one wide
            # tile (the K=64 matmuls sit on disjoint PE row-groups AND
            # disjoint PSUM banks, so they stream concurrently), and one exp
            # drains the whole tile into the pair's P^T tensor. One wide
            # tile per step keeps the ST pipeline 2 steps deep on a 2-buf
            # pool, leaving 4 banks for the acc pool.
            dt = hp
            ptp = pt_pool.tile([P, IT, NIC, 2, NI], BF16, tag="pt")
            st_pair.current = ptp
            fills = list(fills)
            s = 0
            for ic in range(NIC):
                for jt in range(IT):
                    stw = ps_wide.tile([P, 2 * NI], F32, tag="wide", name="stw")
                    for h_off in range(2):
                        r0 = DH * h_off
                        nc.tensor.matmul(
                            stw[:, h_off * NI : (h_off + 1) * NI],
                            kt_sb[r0 : r0 + DH, dt, jt * P : (jt + 1) * P],
                            qt_sb[r0 : r0 + DH, dt, ic * NI : (ic + 1) * NI],
                            start=True,
                            stop=True,
                        )
                    nc.scalar.activation(
                        ptp[:, jt, ic],
                        stw[:],
                        mybir.ActivationFunctionType.Exp,
                        scale=scale,
                    )
                    if s < len(fills):
                        # pending finishers BEFORE this slot's fills, so a
                        # fill chain never re-claims an acc tile whose
                        # normalize is still queued behind the fill's own
                        # PSUM-releasing copy in the in-order DVE stream
                        pending = list(deferred)
                        deferred.clear()
                        for f in pending:
                            f()
                        for f in fills[s]:
                            f()
                    s += 1
            return ptp

        # ---- schedule ----
        mk = lambda f, *a: (lambda: f(*a))

        # Prologue: all four dt0 chains (QT ic0/ic1, KT ic0/ic1) accumulate
        # per-cc in lockstep across the four acc bufs, so every xt/yt chunk
        # is consumed the moment it lands and the prologue ends right after
        # the last input chunk — instead of running four serial chains
        # after the data arrived. Drains on the scalar engine (idle until
        # the first exp); the wv/wo DMA configs queue behind these copies.
        pro = [ps_acc.tile([P, NI], F32, tag="acc", name=f"pro{k}") for k in range(4)]
        for cc in range(CC):
            st0 = cc == 0
            sp1 = cc == CC - 1
            for ic in range(NIC):
                nc.tensor.matmul(
                    pro[ic][:], wq_sb[:, 0, cc * P : (cc + 1) * P],
                    xt_sb[:, cc, ic * NI : (ic + 1) * NI], start=st0, stop=sp1,
                )
                nc.tensor.matmul(
                    pro[2 + ic][:], wk_sb[:, 0, cc * P : (cc + 1) * P],
                    yt_sb[:, cc, ic * NI : (ic + 1) * NI], start=st0, stop=sp1,
                )
        # copy order: the first ST step (ic0, jt0) reads qt-ic0 + kt-ic0
        # only — emit those first so it starts after two copies, not four
        nc.scalar.copy(qt_sb[:, 0, 0:NI], pro[0][:])
        nc.scalar.copy(kt_sb[:, 0, 0:NI], pro[2][:])
        nc.scalar.copy(qt_sb[:, 0, NI : 2 * NI], pro[1][:])
        nc.scalar.copy(kt_sb[:, 0, NI : 2 * NI], pro[3][:])

        # The non-critical 3.5MB (wq/wk dt1..3, wv, wo) is issued from
        # inside the early pair-0 fill slots on the scalar rail: each
        # config lands between two exps, and the transfers only start once
        # the critical 4.5MB has drained — none of it dilutes the startup
        # window. Consumers: QT1/KT1 fills (~+5us), v chains (~+12us),
        # QT2+/wo much later.
        d = lambda o, i: (lambda: nc.scalar.dma_start(out=o, in_=i))
        cfg = [
            d(wq_sb[:, 1], wq[1]),
            d(wk_sb[:, 1], wk[1]),
            d(wv_sb[:, 0:4], wv_r[:, 0:4]),
            d(wv_sb[:, 4:8], wv_r[:, 4:8]),
            d(wq_sb[:, 2], wq[2]),
            d(wk_sb[:, 2], wk[2]),
            d(wq_sb[:, 3], wq[3]),
            d(wk_sb[:, 3], wk[3]),
            d(wo_sb[:], wo.rearrange("(dt p) o -> p dt o", p=P)),
        ]

        # pair 0: non-critical DMA configs in the early slots, QT1/KT1
        # once their dt-blocks land (~+5us), V chains in the back half
        # (wv lands mid-phase); v6/v7 spill into pair 1's first slots.
        q1a = mk(proj_chain, wq_sb, qt_sb, xt_sb, 1, 0)
        q1b = mk(proj_chain, wq_sb, qt_sb, xt_sb, 1, 1)
        k1a = mk(proj_chain, wk_sb, kt_sb, yt_sb, 1, 0)
        k1b = mk(proj_chain, wk_sb, kt_sb, yt_sb, 1, 1)
        pt0 = st_pair(
            0,
            [
                [cfg[0]], [cfg[1]], [cfg[2]], [cfg[3]],
                [q1a, cfg[4]], [cfg[5]], [q1b, cfg[6]], [cfg[7]],
                [k1a, cfg[8]], [k1b],
                [mk(v_chain, 0)], [mk(v_chain, 1)], [mk(v_chain, 2)],
                [mk(v_chain, 3)], [mk(v_chain, 4)], [mk(v_chain, 5)],
            ],
        )

        # pair 1: last V chains, then ctx of heads 0/1 alternating with
        # QT2/KT2
        pt1 = st_pair(
            1,
            [
                [mk(v_chain, 6)], [mk(v_chain, 7)],
                [mk(ctx_chain, 0, pt0, 0)],
                [mk(proj_chain, wq_sb, qt_sb, xt_sb, 2, 0)], [],
                [mk(ctx_chain, 0, pt0, 1)],
                [mk(proj_chain, wq_sb, qt_sb, xt_sb, 2, 1)], [],
                [mk(ctx_chain, 1, pt0, 0)],
                [mk(proj_chain, wk_sb, kt_sb, yt_sb, 2, 0)], [],
                [mk(ctx_chain, 1, pt0, 1)],
                [mk(proj_chain, wk_sb, kt_sb, yt_sb, 2, 1)], [],
                [], [],
            ],
        )

        # pair 2: ctx of heads 2/3 alternating with QT3/KT3
        pt2 = st_pair(
            2,
            [
                [mk(ctx_chain, 2, pt1, 0)], [],
                [mk(proj_chain, wq_sb, qt_sb, xt_sb, 3, 0)], [],
                [mk(ctx_chain, 2, pt1, 1)], [],
                [mk(proj_chain, wq_sb, qt_sb, xt_sb, 3, 1)], [],
                [mk(ctx_chain, 3, pt1, 0)], [],
                [mk(proj_chain, wk_sb, kt_sb, yt_sb, 3, 0)], [],
                [mk(ctx_chain, 3, pt1, 1)], [],
                [mk(proj_chain, wk_sb, kt_sb, yt_sb, 3, 1)], [],
            ],
        )

        # out_a chains over dt 0..2: valid for every it-block once heads
        # 4/5 are normalized (by pair-3 slot 7). Copies alternate
        # scalar/vector (both drain engines have slack mid-phase); DMAs
        # alternate SP/Pool rails so no single rail's ~0.65us-per-config
        # serialization backlogs the kernel tail.
        poA = [
            mk(
                po_chain, it, oc, (0, 1, 2), out_a,
                ("vector", "scalar")[(2 * it + oc) % 2], None,
                (nc.sync, nc.gpsimd)[(2 * it + oc) % 2],
            )
            for it in range(IT)
            for oc in range(NIC)
        ]

        # merged out_c unit for one it-block: both oc halves into one
        # [P, L] staging tile, one 256KB DMA (halves the tail's config
        # count). In-phase units draw two acc tiles; tail units use the
        # two halves of one (by then idle) wide ST tile.
        out_cr = out_c.rearrange("(it p) o -> it p o", p=P)

        def oc_unit(it, p0=None, p1=None):
            if p0 is None:
                p0 = ps_acc.tile([P, NI], F32, tag="acc", name="ocp0")
                p1 = ps_acc.tile([P, NI], F32, tag="acc", name="ocp1")
            for oc, po in ((0, p0), (1, p1)):
                nc.tensor.matmul(
                    po,
                    cx_sb[:, 3, it * P : (it + 1) * P],
                    wo_sb[:, 3, oc * NI : (oc + 1) * NI],
                    start=True,
                    stop=True,
                )
            ot = small.tile([P, L], BF16, tag="ost2", name=f"oc{it}")
            nc.scalar.copy(ot[:, 0:NI], p0)
            nc.vector.tensor_copy(ot[:, NI : 2 * NI], p1)
            (nc.sync, nc.gpsimd)[it % 2].dma_start(out=out_cr[it], in_=ot[:])

        # pair 3: ctx of heads 4/5 first (finishers flushed by slot 7),
        # then — because the ic-outer step order finishes all ic0 exps at
        # mid-phase — ctx of heads 6/7 ic0 runs IN-phase (slots 8/9), their
        # finishers flush by slot 11, and the it0..3 out_c units (which
        # only read the ic0 columns of cx dt3) drain in the last slots.
        # Only heads 6/7 ic1 + it4..7 remain for the tail.
        pt3 = st_pair(
            3,
            [
                [mk(ctx_chain, 4, pt2, 0)], [],
                [mk(ctx_chain, 4, pt2, 1)], [],
                [mk(ctx_chain, 5, pt2, 0)], [],
                [mk(ctx_chain, 5, pt2, 1)], [],
                [mk(ctx_chain, 6, None, 0)],
                [mk(ctx_chain, 7, None, 0), poA[0]],
                poA[1:3],
                poA[3:5],
                poA[5:7] + [mk(oc_unit, 0)],
                poA[7:9] + [mk(oc_unit, 1)],
                poA[9:11] + [mk(oc_unit, 2)],
                poA[11:12] + [mk(oc_unit, 3)],
            ],
        )

        # tail: ctx of heads 6/7 ic1, four out_a chains plugging the
        # normalize-latency window (h6ic1's rs DMA round trip is ~3us),
        # then the it4..7 out_c units on wide-tile halves.
        ctx_chain(6, pt3, 1)
        ctx_chain(7, pt3, 1)
        for f in poA[12:16]:
            f()
        flush_deferred()  # fins for h6ic1 and h7ic1
        for it in range(IT // 2, IT):
            pw = ps_wide.tile([P, 2 * NI], F32, tag="wide", name="po_w")
            oc_unit(it, pw[:, 0:NI], pw[:, NI : 2 * NI])

        if dbg is not None:
            nc.sync.dma_start(out=dbg[0][:], in_=qt_sb[:])
            nc.sync.dma_start(out=dbg[1][:], in_=kt_sb[:])
            nc.sync.dma_start(out=dbg[2][:], in_=va_sb[:])
            nc.sync.dma_start(out=dbg[4][:], in_=cx_sb[:])


_NC_CACHE = None


def _get_nc():
    global _NC_CACHE
    if _NC_CACHE is None:
        _NC_CACHE = _build_kernel()
    return _NC_CACHE


def kernel(x, y, Wq, Wk, Wv, Wo, _trace=False):
    bf = ml_dtypes.bfloat16
    x = np.asarray(x, np.float32)
    y = np.asarray(y, np.float32)
    xtb = [np.ascontiguousarray(np.asarray(x[b]).T).astype(bf) for b in range(B)]
    ytb = [np.ascontiguousarray(np.asarray(y[b]).T).astype(bf) for b in range(B)]
    def _dt_major(w, t):
        # [D, DL] slice -> [DT, P, CC*128]: element (dt, p, cc*128+d) =
        # w[cc*128+p, t*DL + dt*128 + d]  (proj lhsT chunks [P, 128] per
        # (dt, cc), partition dim = contraction rows)
        ws = np.asarray(w)[:, t * DL : (t + 1) * DL]          # [1024, 512]
        ws = ws.reshape(CC, P, DT, P).transpose(2, 1, 0, 3)    # [DT,P,CC,128]
        return np.ascontiguousarray(ws.reshape(DT, P, CC * P)).astype(bf)

    wqs = [_dt_major(Wq, t) for t in range(TP)]
    wks = [_dt_major(Wk, t) for t in range(TP)]
    wvs = [np.ascontiguousarray(np.asarray(Wv)[:, t * DL : (t + 1) * DL]).astype(bf) for t in range(TP)]
    wos = [np.ascontiguousarray(np.asarray(Wo)[t * DL : (t + 1) * DL, :]).astype(bf) for t in range(TP)]

    in_maps = []
    for b in range(B):
        for t in range(TP):
            in_maps.append(
                {
                    "xt": xtb[b],
                    "yt": ytb[b],
                    "wq": wqs[t],
                    "wk": wks[t],
                    "wv": wvs[t],
                    "wo": wos[t],
                }
            )

    nc = _get_nc()
    res = run_bass_kernel_spmd(
        nc, in_maps, core_ids=list(range(N_CORES)), trace=_trace
    )
    out = np.empty((B, L, U), np.float32)
    for b in range(B):
        out[b] = (
            np.asarray(res.results[2 * b]["out_a"], np.float32)
            + np.asarray(res.results[2 * b]["out_c"], np.float32)
            + np.asarray(res.results[2 * b + 1]["out_a"], np.float32)
            + np.asarray(res.results[2 * b + 1]["out_c"], np.float32)
        )
    if _trace:
        return out, res
    return out

